# revision 39
# baseline (speedup 1.0000x reference)
"""DEQ transformer block with Anderson acceleration on 8 Trainium2 NeuronCores.

Sharding: each of the 4 sequences (B=4) is split across a pair of cores
(512 tokens each).  Everything except attention K/V is token-parallel; K/V
halves are exchanged within each pair via AllGather every DEQ iteration.
Activations are feature-major [C, T] in SBUF; matmuls run in fp32r
(full-rate fp32 on the PE, ~13 mantissa bits).
"""

import numpy as np

P = 128
TL = 512          # tokens per core (half a sequence)
C = 768
CCN = 6           # C / 128
NH = 12
DH = 64
HPN = 6           # head pairs
NHID = 3072
HCN = 24          # NHID / 128
KCN = 8           # full-seq key chunks (1024 / 128)
MH = 5            # Anderson history window
LN_EPS = 1e-5
NCORES = 8
GROUPS = [[0, 1], [2, 3], [4, 5], [6, 7]]

_CACHE = {}

TRI = {}
_i = 0
for _a in range(MH):
    for _b in range(_a, MH):
        TRI[(_a, _b)] = _i
        _i += 1


def smat_host(num_iters):
    S = np.zeros((num_iters, 16, 20), np.float32)
    hist = []
    for it in range(num_iters):
        s_new = it % MH
        prev = hist[-4:]
        Kn = len(prev)

        def tri(x, y):
            return TRI[(min(x, y), max(x, y))]

        for a in range(Kn):
            for b in range(Kn):
                col = a * 4 + b
                S[it, tri(prev[a], prev[b]), col] += 1
                S[it, tri(prev[a], s_new), col] -= 1
                S[it, tri(prev[b], s_new), col] -= 1
                S[it, tri(s_new, s_new), col] += 1
            S[it, tri(prev[a], s_new), 16 + a] += 1
            S[it, tri(s_new, s_new), 16 + a] -= 1
        hist.append(s_new)
        if len(hist) > MH:
            hist.pop(0)
    return np.ascontiguousarray(S.transpose(1, 0, 2).reshape(16, num_iters * 20))


def _build(num_iters):
    from contextlib import ExitStack
    import concourse.bass as bass  # noqa
    import concourse.mybir as mybir
    import concourse.tile as tile
    from concourse import bacc
    from concourse.masks import make_identity

    FP = mybir.dt.float32
    FPR = mybir.dt.float32r
    AF = mybir.ActivationFunctionType
    OP = mybir.AluOpType
    AX = mybir.AxisListType

    nc = bacc.Bacc()

    # ---------------- DRAM I/O ----------------
    u_d = nc.dram_tensor("u_fm", [C, TL], FP, kind="ExternalInput")
    qkw_d = nc.dram_tensor("qkw_pack", [12, P, CCN, P], FPR, kind="ExternalInput")
    vw_d = nc.dram_tensor("vw_pack", [CCN, P, C], FPR, kind="ExternalInput")
    wo_d = nc.dram_tensor("wo_pack", [CCN, P, CCN, P], FPR, kind="ExternalInput")
    w1_d = nc.dram_tensor("w1_pack", [HCN, P, CCN, P], FPR, kind="ExternalInput")
    w2_d = nc.dram_tensor("w2t_pack", [HCN, P, CCN, P], FPR, kind="ExternalInput")
    vb_d = nc.dram_tensor("vbias_row", [1, C], FPR, kind="ExternalInput")
    bqk_d = nc.dram_tensor("bqk_cols", [P, 12], FP, kind="ExternalInput")
    bo_d = nc.dram_tensor("bo_cols", [P, CCN], FP, kind="ExternalInput")
    b1_d = nc.dram_tensor("b1_cols", [P, HCN], FP, kind="ExternalInput")
    b2_d = nc.dram_tensor("b2_cols", [P, CCN], FP, kind="ExternalInput")
    ln_d = nc.dram_tensor("ln_cols", [P, 4 * CCN], FP, kind="ExternalInput")
    emb_d = nc.dram_tensor("emb_cols", [P, num_iters * CCN], FP, kind="ExternalInput")
    smat_d = nc.dram_tensor("smat_cols", [16, num_iters * 20], FP,
                            kind="ExternalInput")
    zo_d = nc.dram_tensor("z_out", [C, TL], FP, kind="ExternalOutput")

    # internal DRAM
    # combined K+V exchange buffer (flat): K at [0, C*TL) feature-major,
    # V-even-heads at [VOFF, +TL*384), V-odd-heads at [VOFF2, +TL*384)
    KVN = C * TL + TL * C
    VOFF = C * TL
    VODD = VOFF + TL * HPN * 64
    kvcc = nc.dram_tensor("kv_cc", [KVN], FP)
    kvall = nc.dram_tensor("kv_all", [2, KVN], FP)
    fh = nc.dram_tensor("f_hist", [MH, C, TL], FP)

    with tile.TileContext(nc) as tc:
        ctx = ExitStack()
        pool = ctx.enter_context(tc.tile_pool(name="pers", bufs=1))
        vec = ctx.enter_context(tc.tile_pool(name="vec", bufs=6))
        vrow = ctx.enter_context(tc.tile_pool(name="vrow", bufs=4))
        wpool = ctx.enter_context(tc.tile_pool(name="wpool", bufs=4))
        w2pool = ctx.enter_context(tc.tile_pool(name="w2pool", bufs=2))
        gpool = ctx.enter_context(tc.tile_pool(name="gpool", bufs=2))
        fpool = ctx.enter_context(tc.tile_pool(name="fpool", bufs=4))
        big = ctx.enter_context(tc.tile_pool(name="bigp", bufs=1))
        itp = ctx.enter_context(tc.tile_pool(name="itp", bufs=1))
        pmm = ctx.enter_context(tc.tile_pool(name="pmm", bufs=2, space="PSUM"))
        pscore = ctx.enter_context(tc.tile_pool(name="pscore", bufs=2, space="PSUM"))
        pav = ctx.enter_context(tc.tile_pool(name="pav", bufs=2, space="PSUM"))
        psum2 = ctx.enter_context(tc.tile_pool(name="psum2", bufs=2, space="PSUM"))

        # ------------- persistent tiles -------------
        z_sb = pool.tile([P, CCN, TL], FP, name="z_sb")
        bqk_sb = pool.tile([P, 12], FP, name="bqk_sb")
        bo_sb = pool.tile([P, CCN], FP, name="bo_sb")
        b1_sb = pool.tile([P, HCN], FP, name="b1_sb")
        b2_sb = pool.tile([P, CCN], FP, name="b2_sb")
        ln_sb = pool.tile([P, 4 * CCN], FP, name="ln_sb")
        emb_sb = pool.tile([P, num_iters * CCN], FP, name="emb_sb")
        vb_sb = pool.tile([1, C], FPR, name="vb_sb")
        ident = pool.tile([P, P], FP, name="ident")
        ones1 = pool.tile([P, P], FPR, name="ones1")
        ones2 = pool.tile([P, 2], FPR, name="ones2")
        ones2f = pool.tile([P, 2], FP, name="ones2f")
        coefbc = pool.tile([P, MH, TL], FP, name="coefbc")
        drows = pool.tile([16, TL], FP, name="drows")
        work = pool.tile([P, 4, 28], FP, name="work")
        coef_tm = pool.tile([P, 4, MH], FP, name="coef_tm")
        smat_sb = pool.tile([16, num_iters * 20], FP, name="smat_sb")
        crows = pool.tile([8, TL], FPR, name="crows")

        nc.sync.dma_start(bqk_sb[:], bqk_d[:])
        nc.sync.dma_start(bo_sb[:], bo_d[:])
        nc.sync.dma_start(b1_sb[:], b1_d[:])
        nc.sync.dma_start(b2_sb[:], b2_d[:])
        nc.sync.dma_start(ln_sb[:], ln_d[:])
        nc.sync.dma_start(emb_sb[:], emb_d[:])
        nc.sync.dma_start(vb_sb[:], vb_d[:])
        nc.sync.dma_start(smat_sb[:], smat_d[:])
        make_identity(nc, ident[:])
        nc.vector.memset(drows[:], 0.0)
        onesf = vec.tile([P, P], FP, name="v")
        nc.vector.memset(onesf[:], 1.0)
        nc.scalar.copy(ones1[:], onesf[:])
        nc.scalar.copy(ones2[:], onesf[:, 0:2])
        nc.scalar.copy(ones2f[:], onesf[:, 0:2])

        def ecol(it, cc):
            return emb_sb[:, it * CCN + cc:it * CCN + cc + 1]

        def lncol(which, cc):
            return ln_sb[:, which * CCN + cc:which * CCN + cc + 1]

        TT = nc.vector.tensor_tensor
        TS = nc.vector.tensor_scalar

        def layernorm(src, dst, wb):
            # src/dst: [P, CCN, TL] FPR tiles; wb: 0 for ln1, 2 for ln2
            pmu = pmm.tile([2, TL], FP, name="pg")
            pmsq = pmm.tile([2, TL], FP, name="pg")
            for cc in range(CCN):
                sq = vec.tile([P, TL], FPR, name="v")
                nc.scalar.activation(sq[:], src[:, cc].bitcast(FP), AF.Square)
                nc.tensor.matmul(pmu[:], ones2[:], src[:, cc],
                                 start=(cc == 0), stop=(cc == CCN - 1))
                nc.tensor.matmul(pmsq[:], ones2[:], sq[:],
                                 start=(cc == 0), stop=(cc == CCN - 1))
            mean_r = vrow.tile([1, TL], FPR, name="vr")
            nc.scalar.activation(mean_r[:], pmu[0:1, :], AF.Identity, scale=1.0 / C)
            msq_r = vrow.tile([1, TL], FP, name="vr")
            nc.scalar.activation(msq_r[:], pmsq[0:1, :], AF.Identity, scale=1.0 / C)
            var_r = vrow.tile([1, TL], FP, name="vr")
            TT(out=var_r[:], in0=mean_r[:].bitcast(FP), in1=mean_r[:].bitcast(FP),
               op=OP.mult)
            TT(out=var_r[:], in0=msq_r[:], in1=var_r[:], op=OP.subtract)
            TS(out=var_r[:], in0=var_r[:], scalar1=LN_EPS, scalar2=None,
               op0=OP.add)
            sd_r = vrow.tile([1, TL], FP, name="vr")
            nc.scalar.activation(sd_r[:], var_r[:], AF.Sqrt)
            rstd_f = vrow.tile([1, TL], FP, name="vr")
            nc.vector.reciprocal_approx_fast(rstd_f[:], sd_r[:])
            rstd_r = vrow.tile([1, TL], FPR, name="vr")
            nc.scalar.copy(rstd_r[:], rstd_f[:])
            pmean = pscore.tile([P, TL], FP, name="sc")
            prstd = pscore.tile([P, TL], FP, name="sc")
            nc.tensor.matmul(pmean[:], ones1[0:1, :], mean_r[:], start=True, stop=True)
            nc.tensor.matmul(prstd[:], ones1[0:1, :], rstd_r[:],
                             start=True, stop=True)
            pmean_sb = vec.tile([P, TL], FP, name="v")
            prstd_sb = vec.tile([P, TL], FP, name="v")
            nc.scalar.copy(pmean_sb[:], pmean[:])
            nc.scalar.copy(prstd_sb[:], prstd[:])
            for cc in range(CCN):
                t1 = vec.tile([P, TL], FP, name="v")
                TT(out=t1[:], in0=src[:, cc].bitcast(FP),
                   in1=pmean_sb[:], op=OP.subtract)
                TT(out=t1[:], in0=t1[:], in1=prstd_sb[:], op=OP.mult)
                TS(out=dst[:, cc], in0=t1[:], scalar1=lncol(wb, cc),
                   scalar2=lncol(wb + 1, cc), op0=OP.mult, op1=OP.add)

        hist = []  # slot ids of stored residuals, oldest..newest

        for it in range(num_iters):
            s_new = it % MH
            prev = hist[-4:]
            Kn = len(prev)

            zctx = itp.tile([P, CCN, TL], FPR, name="zctx")
            x1 = itp.tile([P, CCN, TL], FPR, name="xln")
            q_sb = itp.tile([P, CCN, TL], FPR, name="qattn")
            k_loc = itp.tile([P, CCN, TL], FPR, name="k_loc")
            k_rem = itp.tile([P, CCN, TL], FPR, name="k_rem")
            # V layout: [kc, head-pair, 130]: even head v at 0:64, ones at
            # col 64 (even head softmax denominator rides the AV matmul as
            # psum row 64), odd head v at 65:129, ones at col 129
            v_loc = itp.tile([P, 4, HPN, 130], FPR, name="v_loc")
            v_rem = itp.tile([P, 4, HPN, 130], FPR, name="v_rem")
            out_fm = itp.tile([P, CCN, TL], FPR, name="zctx")
            nc.vector.memset(v_loc[:].bitcast(FP), 1.0)
            nc.vector.memset(v_rem[:].bitcast(FP), 1.0)

            # ---- A1: z_ctx = z + u + 0.1*emb_it ; x1 = LN1(z_ctx) ----
            for cc in range(CCN):
                ut = fpool.tile([P, TL], FP, name="ft")
                nc.sync.dma_start(ut[:], u_d[cc * P:(cc + 1) * P, :])
                if it == 0:
                    TS(out=zctx[:, cc], in0=ut[:], scalar1=ecol(it, cc),
                       scalar2=None, op0=OP.add)
                else:
                    t0 = vec.tile([P, TL], FP, name="v")
                    TS(out=t0[:], in0=z_sb[:, cc], scalar1=ecol(it, cc),
                       scalar2=None, op0=OP.add)
                    TT(out=zctx[:, cc], in0=t0[:], in1=ut[:], op=OP.add)

            layernorm(zctx, x1, 0)

            # ---- A2: K (feature-major) and V (token-major) projections ----
            for oc in range(CCN):
                wt = wpool.tile([P, CCN, P], FPR, name="wt")
                nc.sync.dma_start(wt[:], qkw_d[6 + oc])
                pk = pmm.tile([P, TL], FP, name="pg")
                for cc in range(CCN):
                    nc.tensor.matmul(pk[:], wt[:, cc], x1[:, cc],
                                     start=(cc == 0), stop=(cc == CCN - 1))
                nc.scalar.activation(k_loc[:, oc], pk[:], AF.Identity,
                                     bias=bqk_sb[:, 6 + oc:7 + oc])
                nc.sync.dma_start(
                    kvcc[oc * P * TL:(oc + 1) * P * TL],
                    k_loc[:, oc].bitcast(FP))

            vw = big.tile([P, CCN, C], FPR, name="bigt")
            for cc in range(CCN):
                nc.sync.dma_start(vw[:, cc], vw_d[cc])
            for tch in range(4):
                pva = pmm.tile([P, 4, 2, 64], FP, name="pg")
                pvb = pmm.tile([P, 2, 2, 64], FP, name="pg")
                ts = slice(tch * P, (tch + 1) * P)
                for cc in range(CCN):
                    nc.tensor.matmul(pva[:], x1[:, cc, ts], vw[:, cc, 0:512],
                                     start=(cc == 0), stop=False)
                    nc.tensor.matmul(pvb[:], x1[:, cc, ts],
                                     vw[:, cc, 512:768],
                                     start=(cc == 0), stop=False)
                nc.tensor.matmul(pva[:], ones1[0:1, :], vb_sb[:, 0:512],
                                 start=False, stop=True)
                nc.tensor.matmul(pvb[:], ones1[0:1, :], vb_sb[:, 512:768],
                                 start=False, stop=True)
                nc.vector.tensor_copy(v_loc[:, tch, 0:4, 0:64],
                                      pva[:, :, 0, :])
                nc.vector.tensor_copy(v_loc[:, tch, 0:4, 65:129],
                                      pva[:, :, 1, :])
                nc.vector.tensor_copy(v_loc[:, tch, 4:6, 0:64],
                                      pvb[:, :, 0, :])
                nc.vector.tensor_copy(v_loc[:, tch, 4:6, 65:129],
                                      pvb[:, :, 1, :])
                nc.sync.dma_start(
                    kvcc[VOFF + tch * P * 384:VOFF + (tch + 1) * P * 384],
                    v_loc[:, tch, :, 0:64].bitcast(FP))
                nc.sync.dma_start(
                    kvcc[VODD + tch * P * 384:VODD + (tch + 1) * P * 384],
                    v_loc[:, tch, :, 65:129].bitcast(FP))

            nc.gpsimd.collective_compute(
                "AllGather", OP.bypass, replica_groups=GROUPS,
                ins=[kvcc[:]], outs=[kvall[:]])

            # ---- A3: Q projection (overlaps the V collective) ----
            for oc in range(CCN):
                wt = wpool.tile([P, CCN, P], FPR, name="wt")
                nc.sync.dma_start(wt[:], qkw_d[oc])
                pq = pmm.tile([P, TL], FP, name="pg")
                for cc in range(CCN):
                    nc.tensor.matmul(pq[:], wt[:, cc], x1[:, cc],
                                     start=(cc == 0), stop=(cc == CCN - 1))
                nc.scalar.activation(q_sb[:, oc], pq[:], AF.Identity,
                                     bias=bqk_sb[:, oc:oc + 1])

            # remote K/V = gathered slot0 + slot1 - local (rank-agnostic)
            for r in range(2):
                eng = nc.sync if r == 0 else nc.gpsimd
                for cc in range(CCN):
                    kw = {} if r == 0 else {"accum_op": OP.add}
                    eng.dma_start(
                        k_rem[:, cc].bitcast(FP),
                        kvall[r, cc * P * TL:(cc + 1) * P * TL], **kw)
                for tch in range(4):
                    kw = {} if r == 0 else {"accum_op": OP.add}
                    eng.dma_start(
                        v_rem[:, tch, :, 0:64].bitcast(FP),
                        kvall[r, VOFF + tch * P * 384:
                              VOFF + (tch + 1) * P * 384], **kw)
                    eng.dma_start(
                        v_rem[:, tch, :, 65:129].bitcast(FP),
                        kvall[r, VODD + tch * P * 384:
                              VODD + (tch + 1) * P * 384], **kw)
            TT(out=k_rem[:], in0=k_rem[:].bitcast(FP),
               in1=k_loc[:].bitcast(FP), op=OP.subtract)
            for tch in range(4):
                TT(out=v_rem[:, tch, :, 0:64],
                   in0=v_rem[:, tch, :, 0:64].bitcast(FP),
                   in1=v_loc[:, tch, :, 0:64].bitcast(FP), op=OP.subtract)
                TT(out=v_rem[:, tch, :, 65:129],
                   in0=v_rem[:, tch, :, 65:129].bitcast(FP),
                   in1=v_loc[:, tch, :, 65:129].bitcast(FP), op=OP.subtract)

            # ---- B: attention (softmax denominator rides in the AV matmul
            # via the ones columns of v_sb) ----
            for hp in range(HPN):
                pava = pav.tile([P, TL], FP, name="pv")
                pavb = pav.tile([P, TL], FP, name="pv")
                for kc in range(KCN):
                    if kc < 4:
                        kt, vt, ks = k_loc, v_loc, slice(kc * P, (kc + 1) * P)
                        vkc = kc
                    else:
                        kt, vt = k_rem, v_rem
                        ks = slice((kc - 4) * P, (kc - 3) * P)
                        vkc = kc - 4
                    sca = pscore.tile([P, TL], FP, name="sc")
                    scb = pscore.tile([P, TL], FP, name="sc")
                    nc.tensor.matmul(sca[:], kt[0:64, hp, ks], q_sb[0:64, hp],
                                     start=True, stop=True)
                    nc.tensor.matmul(scb[:], kt[64:128, hp, ks],
                                     q_sb[64:128, hp], start=True, stop=True)
                    atta = vec.tile([P, TL], FPR, name="v")
                    attb = vec.tile([P, TL], FPR, name="v")
                    nc.scalar.activation(atta[:], sca[:], AF.Exp, scale=0.125)
                    nc.scalar.activation(attb[:], scb[:], AF.Exp, scale=0.125)
                    nc.tensor.matmul(pava[0:65, :], vt[:, vkc, hp, 0:65],
                                     atta[:], start=(kc == 0),
                                     stop=(kc == KCN - 1))
                    nc.tensor.matmul(pavb[0:65, :], vt[:, vkc, hp, 65:130],
                                     attb[:], start=(kc == 0),
                                     stop=(kc == KCN - 1))
                rar = vec.tile([P, TL], FPR, name="v")
                rbr = vec.tile([P, TL], FPR, name="v")
                with nc.allow_low_precision(reason="fp32r for PE broadcast"):
                    nc.vector.reciprocal(rar[64:65, :], pava[64:65, :])
                    nc.vector.reciprocal(rbr[64:65, :], pavb[64:65, :])
                pba = pscore.tile([P, TL], FP, name="sc")
                pbb = pscore.tile([P, TL], FP, name="sc")
                nc.tensor.matmul(pba[0:64, :], ones1[64:65, 0:64],
                                 rar[64:65, :],
                                 start=True, stop=True)
                nc.tensor.matmul(pbb[0:64, :], ones1[64:65, 0:64],
                                 rbr[64:65, :], start=True, stop=True)
                bc_sb = vec.tile([P, TL], FP, name="v")
                bcb_sb = vec.tile([P, TL], FP, name="v")
                nc.scalar.copy(bc_sb[0:64, :], pba[0:64, :])
                nc.scalar.copy(bcb_sb[0:64, :], pbb[0:64, :])
                TT(out=out_fm[0:64, hp], in0=pava[0:64, :], in1=bc_sb[0:64, :],
                   op=OP.mult)
                tb = vec.tile([64, TL], FPR, name="vtb")
                TT(out=tb[:], in0=pavb[0:64, :], in1=bcb_sb[0:64, :],
                   op=OP.mult)
                nc.sync.dma_start(out_fm[64:128, hp], tb[:])

            # ---- C: output projection -> attnres (f32) ----
            attnres = itp.tile([P, CCN, TL], FP, name="qattn")
            for oc in range(CCN):
                wt = wpool.tile([P, CCN, P], FPR, name="wt")
                nc.sync.dma_start(wt[:], wo_d[oc])
                pp = pmm.tile([P, TL], FP, name="pg")
                for ci in range(CCN):
                    nc.tensor.matmul(pp[:], wt[:, ci], out_fm[:, ci],
                                     start=(ci == 0), stop=(ci == CCN - 1))
                nc.scalar.activation(attnres[:, oc], pp[:], AF.Identity,
                                     bias=bo_sb[:, oc:oc + 1])

            # ---- D: z_attn = z + attnres ; x2 = LN2(z_attn) ----
            za = itp.tile([P, CCN, TL], FPR, name="zctx")
            for cc in range(CCN):
                if it == 0:
                    nc.vector.tensor_copy(za[:, cc], attnres[:, cc])
                else:
                    TT(out=za[:, cc], in0=z_sb[:, cc], in1=attnres[:, cc],
                       op=OP.add)
            x2 = itp.tile([P, CCN, TL], FPR, name="xln")
            layernorm(za, x2, 2)

            # ---- E: MLP fused per hidden-block; res += mlp into attnres ----
            po = [pmm.tile([P, TL], FP, name="pg"),
                  pmm.tile([P, TL], FP, name="pg"),
                  pscore.tile([P, TL], FP, name="sc"),
                  pscore.tile([P, TL], FP, name="sc"),
                  pav.tile([P, TL], FP, name="pv"),
                  pav.tile([P, TL], FP, name="pv")]
            for hi in range(HCN):
                w1t = wpool.tile([P, CCN, P], FPR, name="wt")
                nc.sync.dma_start(w1t[:], w1_d[hi])
                w2t = w2pool.tile([P, CCN, P], FPR, name="w2t")
                nc.sync.dma_start(w2t[:], w2_d[hi])
                ph = psum2.tile([P, TL], FP, name="p2")
                for cc in range(CCN):
                    nc.tensor.matmul(ph[:], w1t[:, cc], x2[:, cc],
                                     start=(cc == 0), stop=(cc == CCN - 1))
                g = gpool.tile([P, TL], FPR, name="g")
                nc.scalar.activation(g[:], ph[:], AF.Gelu,
                                     bias=b1_sb[:, hi:hi + 1])
                for oc in range(CCN):
                    nc.tensor.matmul(po[oc][:], w2t[:, oc], g[:],
                                     start=(hi == 0), stop=(hi == HCN - 1))
            for oc in range(CCN):
                t2 = vec.tile([P, TL], FP, name="v")
                TS(out=t2[:], in0=po[oc][:],
                   scalar1=b2_sb[:, oc:oc + 1], scalar2=None, op0=OP.add)
                TT(out=attnres[:, oc], in0=attnres[:, oc],
                   in1=t2[:], op=OP.add)

            # store res as newest history entry
            for cc in range(CCN):
                nc.sync.dma_start(fh[s_new, cc * P:(cc + 1) * P, :], attnres[:, cc])

            # ---- F: Anderson update ----
            # raw-dot cache: drows row TRI[(a,b)] = per-token <F_a, F_b>
            # (slots a<=b).  Each iteration adds Kn+1 new dot rows (history
            # slots vs the fresh residual + its self-dot); the Gram matrix /
            # rhs of the per-token least squares is then assembled from
            # cached rows by one constant-matrix f32 matmul (smat).
            pdl_alloc = [(pmm, "pg"), (pmm, "pg"), (pscore, "sc"),
                         (pscore, "sc"), (pav, "pv")]
            pdl = []
            for k in range(Kn + 1):
                pl, nm = pdl_alloc[k]
                pdl.append(pl.tile([2, TL], FP, name=nm))
            for cc in range(CCN):
                for k in range(Kn):
                    ft = fpool.tile([P, TL], FP, name="ft")
                    nc.sync.dma_start(
                        ft[:], fh[prev[k], cc * P:(cc + 1) * P, :])
                    prod = vec.tile([P, TL], FP, name="v")
                    TT(out=prod[:], in0=ft[:], in1=attnres[:, cc], op=OP.mult)
                    nc.tensor.matmul(pdl[k][:], ones2f[:], prod[:],
                                     start=(cc == 0), stop=(cc == CCN - 1))
                sqp = vec.tile([P, TL], FP, name="v")
                nc.scalar.activation(sqp[:], attnres[:, cc], AF.Square)
                nc.tensor.matmul(pdl[Kn][:], ones2f[:], sqp[:],
                                 start=(cc == 0), stop=(cc == CCN - 1))
            for k in range(Kn + 1):
                os = prev[k] if k < Kn else s_new
                row = TRI[(min(os, s_new), max(os, s_new))]
                stage = vrow.tile([1, TL], FP, name="vr")
                nc.scalar.copy(stage[:], pdl[k][0:1, :])
                nc.sync.dma_start(drows[row:row + 1, :], stage[:])

            if Kn == 0:
                for cc in range(CCN):
                    nc.vector.tensor_copy(z_sb[:, cc], attnres[:, cc])
            else:
                # G grid (4x4 incl. symmetric dups) + rhs rows from the
                # dot cache, then transpose to token-major work layout
                gps = psum2.tile([P, TL], FP, name="p2")
                nc.tensor.matmul(gps[0:20, :],
                                 smat_sb[:, it * 20:(it + 1) * 20],
                                 drows[:], start=True, stop=True)
                gsb = vec.tile([P, TL], FP, name="v")
                nc.scalar.copy(gsb[0:20, :], gps[0:20, :])
                for tch in range(4):
                    ptr = pmm.tile([P, TL], FP, name="pg")
                    nc.tensor.transpose(ptr[:, 0:20],
                                        gsb[0:20, tch * P:(tch + 1) * P],
                                        ident[0:20, 0:20])
                    nc.scalar.copy(work[:, tch, 0:20], ptr[:, 0:20])
                for a in range(Kn):
                    TS(out=work[:, :, a * 4 + a], in0=work[:, :, a * 4 + a],
                       scalar1=1e-6, scalar2=None, op0=OP.add)

                def As(a, b):
                    return work[:, :, a * 4 + b]

                def Bs(k):
                    return work[:, :, 16 + k]

                def Al(k):
                    return work[:, :, 20 + k]

                rin = work[:, :, 24]
                tmp = work[:, :, 25]
                fco = work[:, :, 26]
                for i in range(Kn):
                    nc.vector.reciprocal_approx_fast(rin, As(i, i))
                    for j in range(i + 1, Kn):
                        TT(out=fco, in0=As(j, i), in1=rin, op=OP.mult)
                        for m in range(i, Kn):
                            TT(out=tmp, in0=fco, in1=As(i, m), op=OP.mult)
                            TT(out=As(j, m), in0=As(j, m), in1=tmp,
                               op=OP.subtract)
                        TT(out=tmp, in0=fco, in1=Bs(i), op=OP.mult)
                        TT(out=Bs(j), in0=Bs(j), in1=tmp, op=OP.subtract)
                for i in range(Kn - 1, -1, -1):
                    nc.vector.tensor_copy(tmp, Bs(i))
                    for j in range(i + 1, Kn):
                        TT(out=fco, in0=As(i, j), in1=Al(j), op=OP.mult)
                        TT(out=tmp, in0=tmp, in1=fco, op=OP.subtract)
                    nc.vector.reciprocal_approx_fast(rin, As(i, i))
                    TT(out=Al(i), in0=tmp, in1=rin, op=OP.mult)

                # coeffs: col0 = 1 + sum(alpha); cols 1..Kn = -alpha
                if Kn == 1:
                    TS(out=coef_tm[:, :, 0], in0=Al(0), scalar1=1.0,
                       scalar2=None, op0=OP.add)
                else:
                    nc.vector.tensor_reduce(out=coef_tm[:, :, 0:1],
                                            in_=work[:, :, 20:20 + Kn],
                                            axis=AX.X, op=OP.add)
                    TS(out=coef_tm[:, :, 0], in0=coef_tm[:, :, 0],
                       scalar1=1.0, scalar2=None, op0=OP.add)
                TS(out=coef_tm[:, :, 1:1 + Kn], in0=work[:, :, 20:20 + Kn],
                   scalar1=-1.0, scalar2=None, op0=OP.mult)

                for tch in range(4):
                    ptr = pmm.tile([P, TL], FP, name="pg")
                    nc.tensor.transpose(ptr[0:Kn + 1, 0:P],
                                        coef_tm[:, tch, 0:Kn + 1], ident[:])
                    nc.scalar.copy(crows[0:Kn + 1, tch * P:(tch + 1) * P],
                                   ptr[0:Kn + 1, 0:P])

                for k in range(Kn + 1):
                    cst = vrow.tile([1, TL], FPR, name="vrr")
                    nc.sync.dma_start(cst[:], crows[k:k + 1, :])
                    pb = pscore.tile([P, TL], FP, name="sc")
                    nc.tensor.matmul(pb[:], ones1[0:1, :], cst[:],
                                     start=True, stop=True)
                    nc.scalar.copy(coefbc[:, k, :], pb[:])

                # z += c0*res + sum_k ck*F_k
                for cc in range(CCN):
                    t0 = vec.tile([P, TL], FP, name="v")
                    TT(out=t0[:], in0=attnres[:, cc], in1=coefbc[:, 0, :],
                       op=OP.mult)
                    TT(out=z_sb[:, cc], in0=z_sb[:, cc], in1=t0[:], op=OP.add)
                    for k in range(Kn):
                        ft = fpool.tile([P, TL], FP, name="ft")
                        nc.sync.dma_start(
                            ft[:], fh[prev[k], cc * P:(cc + 1) * P, :])
                        TT(out=t0[:], in0=ft[:], in1=coefbc[:, k + 1, :],
                           op=OP.mult)
                        TT(out=z_sb[:, cc], in0=z_sb[:, cc], in1=t0[:],
                           op=OP.add)

            hist.append(s_new)
            if len(hist) > MH:
                hist.pop(0)

        for cc in range(CCN):
            nc.sync.dma_start(zo_d[cc * P:(cc + 1) * P, :], z_sb[:, cc])

        ctx.close()

    nc.finalize()
    return nc


def _host_pack(inputs, num_iters):
    f32 = np.float32
    ipw = np.ascontiguousarray(inputs["in_proj_w"], f32)
    ipb = np.ascontiguousarray(inputs["in_proj_b"], f32)
    opw = np.ascontiguousarray(inputs["out_proj_w"], f32)
    opb = np.ascontiguousarray(inputs["out_proj_b"], f32)
    w1 = np.ascontiguousarray(inputs["mlp_w1"], f32)
    b1 = np.ascontiguousarray(inputs["mlp_b1"], f32)
    w2 = np.ascontiguousarray(inputs["mlp_w2"], f32)
    b2 = np.ascontiguousarray(inputs["mlp_b2"], f32)
    emb = np.ascontiguousarray(inputs["iter_emb"], f32)

    qkw_pack = np.ascontiguousarray(
        ipw[:1536].reshape(12, P, CCN, P).transpose(0, 3, 2, 1))
    vw_pack = np.ascontiguousarray(ipw[1536:].T.reshape(CCN, P, C))
    wo_pack = np.ascontiguousarray(
        opw.reshape(CCN, P, CCN, P).transpose(0, 3, 2, 1))
    w1_pack = np.ascontiguousarray(
        w1.reshape(HCN, P, CCN, P).transpose(0, 3, 2, 1))
    w2t_pack = np.ascontiguousarray(
        w2.T.reshape(HCN, P, CCN, P))
    vbias_row = np.ascontiguousarray(ipb[1536:].reshape(1, C))
    bqk_cols = np.ascontiguousarray(ipb[:1536].reshape(12, P).T)
    bo_cols = np.ascontiguousarray(opb.reshape(CCN, P).T)
    b1_cols = np.ascontiguousarray(b1.reshape(HCN, P).T)
    b2_cols = np.ascontiguousarray(b2.reshape(CCN, P).T)
    ln_cols = np.ascontiguousarray(np.stack(
        [inputs["ln1_w"], inputs["ln1_b"], inputs["ln2_w"], inputs["ln2_b"]],
        0).astype(f32).reshape(4 * CCN, P).T)
    smat_cols = smat_host(num_iters)
    rows = [min(i, emb.shape[0] - 1) for i in range(num_iters)]
    emb_cols = np.ascontiguousarray(
        (0.1 * emb[rows]).reshape(num_iters, CCN, P).transpose(2, 0, 1)
        .reshape(P, num_iters * CCN))
    shared = dict(
        qkw_pack=qkw_pack, vw_pack=vw_pack, wo_pack=wo_pack, w1_pack=w1_pack,
        w2t_pack=w2t_pack, vbias_row=vbias_row, bqk_cols=bqk_cols,
        bo_cols=bo_cols, b1_cols=b1_cols, b2_cols=b2_cols, ln_cols=ln_cols,
        emb_cols=emb_cols, smat_cols=smat_cols)
    u = np.ascontiguousarray(inputs["u"], f32)
    in_maps = []
    for core in range(NCORES):
        b, h = core // 2, core % 2
        m = dict(shared)
        m["u_fm"] = np.ascontiguousarray(u[b, h * TL:(h + 1) * TL, :].T)
        in_maps.append(m)
    return in_maps


def run_device(inputs, num_iters=None, trace=False):
    from concourse.bass_utils import run_bass_kernel_spmd
    ni = int(inputs.get("num_iters", 6)) if num_iters is None else num_iters
    if ni not in _CACHE:
        _CACHE[ni] = _build(ni)
    nc = _CACHE[ni]
    in_maps = _host_pack(inputs, ni)
    r = run_bass_kernel_spmd(nc, in_maps, list(range(NCORES)), trace=trace)
    u = inputs["u"]
    B, T, _ = u.shape
    out = np.empty((B, T, C), np.float32)
    for core in range(NCORES):
        b, h = core // 2, core % 2
        out[b, h * TL:(h + 1) * TL, :] = r.results[core]["z_out"].T
    return out, r


def kernel(**inputs):
    out, _ = run_device(inputs)
    return out.astype(np.float32)



# revision 40
# speedup vs baseline: 1.0168x; 1.0168x over previous
"""DEQ transformer block with Anderson acceleration on 8 Trainium2 NeuronCores.

Sharding: each of the 4 sequences (B=4) is split across a pair of cores
(512 tokens each).  Everything except attention K/V is token-parallel; K/V
halves are exchanged within each pair via AllGather every DEQ iteration.
Activations are feature-major [C, T] in SBUF; matmuls run in fp32r
(full-rate fp32 on the PE, ~13 mantissa bits).
"""

import numpy as np

P = 128
TL = 512          # tokens per core (half a sequence)
C = 768
CCN = 6           # C / 128
NH = 12
DH = 64
HPN = 6           # head pairs
NHID = 3072
HCN = 24          # NHID / 128
KCN = 8           # full-seq key chunks (1024 / 128)
MH = 5            # Anderson history window
LN_EPS = 1e-5
NCORES = 8
GROUPS = [[0, 1], [2, 3], [4, 5], [6, 7]]

_CACHE = {}

TRI = {}
_i = 0
for _a in range(MH):
    for _b in range(_a, MH):
        TRI[(_a, _b)] = _i
        _i += 1


def smat_host(num_iters):
    S = np.zeros((num_iters, 16, 20), np.float32)
    hist = []
    for it in range(num_iters):
        s_new = it % MH
        prev = hist[-4:]
        Kn = len(prev)

        def tri(x, y):
            return TRI[(min(x, y), max(x, y))]

        for a in range(Kn):
            for b in range(Kn):
                col = a * 4 + b
                S[it, tri(prev[a], prev[b]), col] += 1
                S[it, tri(prev[a], s_new), col] -= 1
                S[it, tri(prev[b], s_new), col] -= 1
                S[it, tri(s_new, s_new), col] += 1
            S[it, tri(prev[a], s_new), 16 + a] += 1
            S[it, tri(s_new, s_new), 16 + a] -= 1
        hist.append(s_new)
        if len(hist) > MH:
            hist.pop(0)
    return np.ascontiguousarray(S.transpose(1, 0, 2).reshape(16, num_iters * 20))


def _build(num_iters):
    from contextlib import ExitStack
    import concourse.bass as bass  # noqa
    import concourse.mybir as mybir
    import concourse.tile as tile
    from concourse import bacc
    from concourse.masks import make_identity

    FP = mybir.dt.float32
    FPR = mybir.dt.float32r
    AF = mybir.ActivationFunctionType
    OP = mybir.AluOpType
    AX = mybir.AxisListType

    nc = bacc.Bacc()

    # ---------------- DRAM I/O ----------------
    u_d = nc.dram_tensor("u_fm", [C, TL], FP, kind="ExternalInput")
    qkw_d = nc.dram_tensor("qkw_pack", [12, P, CCN, P], FPR, kind="ExternalInput")
    vw_d = nc.dram_tensor("vw_pack", [CCN, P, C], FPR, kind="ExternalInput")
    wo_d = nc.dram_tensor("wo_pack", [CCN, P, CCN, P], FPR, kind="ExternalInput")
    w1_d = nc.dram_tensor("w1_pack", [HCN, P, CCN, P], FPR, kind="ExternalInput")
    w2_d = nc.dram_tensor("w2t_pack", [HCN, P, CCN, P], FPR, kind="ExternalInput")
    vb_d = nc.dram_tensor("vbias_row", [1, C], FPR, kind="ExternalInput")
    bqk_d = nc.dram_tensor("bqk_cols", [P, 12], FP, kind="ExternalInput")
    bo_d = nc.dram_tensor("bo_cols", [P, CCN], FP, kind="ExternalInput")
    b1_d = nc.dram_tensor("b1_cols", [P, HCN], FP, kind="ExternalInput")
    b2_d = nc.dram_tensor("b2_cols", [P, CCN], FP, kind="ExternalInput")
    ln_d = nc.dram_tensor("ln_cols", [P, 4 * CCN], FP, kind="ExternalInput")
    emb_d = nc.dram_tensor("emb_cols", [P, num_iters * CCN], FP, kind="ExternalInput")
    smat_d = nc.dram_tensor("smat_cols", [16, num_iters * 20], FP,
                            kind="ExternalInput")
    zo_d = nc.dram_tensor("z_out", [C, TL], FP, kind="ExternalOutput")

    # internal DRAM
    # combined K+V exchange buffer (flat): K at [0, C*TL) feature-major,
    # V-even-heads at [VOFF, +TL*384), V-odd-heads at [VOFF2, +TL*384)
    KVN = C * TL + TL * C
    VOFF = C * TL
    VODD = VOFF + TL * HPN * 64
    kvcc = nc.dram_tensor("kv_cc", [KVN], FP)
    kvall = nc.dram_tensor("kv_all", [2, KVN], FP)
    fh = nc.dram_tensor("f_hist", [MH, C, TL], FP)

    with tile.TileContext(nc) as tc:
        ctx = ExitStack()
        pool = ctx.enter_context(tc.tile_pool(name="pers", bufs=1))
        vec = ctx.enter_context(tc.tile_pool(name="vec", bufs=6))
        vrow = ctx.enter_context(tc.tile_pool(name="vrow", bufs=4))
        wpool = ctx.enter_context(tc.tile_pool(name="wpool", bufs=4))
        w2pool = ctx.enter_context(tc.tile_pool(name="w2pool", bufs=2))
        gpool = ctx.enter_context(tc.tile_pool(name="gpool", bufs=2))
        fpool = ctx.enter_context(tc.tile_pool(name="fpool", bufs=4))
        big = ctx.enter_context(tc.tile_pool(name="bigp", bufs=1))
        itp = ctx.enter_context(tc.tile_pool(name="itp", bufs=1))
        pmm = ctx.enter_context(tc.tile_pool(name="pmm", bufs=2, space="PSUM"))
        pscore = ctx.enter_context(tc.tile_pool(name="pscore", bufs=2, space="PSUM"))
        pav = ctx.enter_context(tc.tile_pool(name="pav", bufs=2, space="PSUM"))
        psum2 = ctx.enter_context(tc.tile_pool(name="psum2", bufs=2, space="PSUM"))

        # ------------- persistent tiles -------------
        z_sb = pool.tile([P, CCN, TL], FP, name="z_sb")
        bqk_sb = pool.tile([P, 12], FP, name="bqk_sb")
        bo_sb = pool.tile([P, CCN], FP, name="bo_sb")
        b1_sb = pool.tile([P, HCN], FP, name="b1_sb")
        b2_sb = pool.tile([P, CCN], FP, name="b2_sb")
        ln_sb = pool.tile([P, 4 * CCN], FP, name="ln_sb")
        emb_sb = pool.tile([P, num_iters * CCN], FP, name="emb_sb")
        vb_sb = pool.tile([1, C], FPR, name="vb_sb")
        ident = pool.tile([P, P], FP, name="ident")
        ones1 = pool.tile([P, P], FPR, name="ones1")
        ones2 = pool.tile([P, 2], FPR, name="ones2")
        ones2f = pool.tile([P, 2], FP, name="ones2f")
        coefbc = pool.tile([P, MH, TL], FP, name="coefbc")
        drows = pool.tile([16, TL], FP, name="drows")
        work = pool.tile([P, 4, 28], FP, name="work")
        coef_tm = pool.tile([P, 4, MH], FP, name="coef_tm")
        smat_sb = pool.tile([16, num_iters * 20], FP, name="smat_sb")
        crows = pool.tile([8, TL], FPR, name="crows")

        nc.sync.dma_start(bqk_sb[:], bqk_d[:])
        nc.sync.dma_start(bo_sb[:], bo_d[:])
        nc.sync.dma_start(b1_sb[:], b1_d[:])
        nc.sync.dma_start(b2_sb[:], b2_d[:])
        nc.sync.dma_start(ln_sb[:], ln_d[:])
        nc.sync.dma_start(emb_sb[:], emb_d[:])
        nc.sync.dma_start(vb_sb[:], vb_d[:])
        nc.sync.dma_start(smat_sb[:], smat_d[:])
        make_identity(nc, ident[:])
        nc.vector.memset(drows[:], 0.0)
        onesf = vec.tile([P, P], FP, name="v")
        nc.vector.memset(onesf[:], 1.0)
        nc.scalar.copy(ones1[:], onesf[:])
        nc.scalar.copy(ones2[:], onesf[:, 0:2])
        nc.scalar.copy(ones2f[:], onesf[:, 0:2])

        def ecol(it, cc):
            return emb_sb[:, it * CCN + cc:it * CCN + cc + 1]

        def lncol(which, cc):
            return ln_sb[:, which * CCN + cc:which * CCN + cc + 1]

        TT = nc.vector.tensor_tensor
        TS = nc.vector.tensor_scalar

        def layernorm(src, dst, wb):
            # src/dst: [P, CCN, TL] FPR tiles; wb: 0 for ln1, 2 for ln2
            pmu = pmm.tile([2, TL], FP, name="pg")
            pmsq = pmm.tile([2, TL], FP, name="pg")
            for cc in range(CCN):
                sq = vec.tile([P, TL], FPR, name="v")
                nc.scalar.activation(sq[:], src[:, cc].bitcast(FP), AF.Square)
                nc.tensor.matmul(pmu[:], ones2[:], src[:, cc],
                                 start=(cc == 0), stop=(cc == CCN - 1))
                nc.tensor.matmul(pmsq[:], ones2[:], sq[:],
                                 start=(cc == 0), stop=(cc == CCN - 1))
            mean_r = vrow.tile([1, TL], FPR, name="vr")
            nc.scalar.activation(mean_r[:], pmu[0:1, :], AF.Identity, scale=1.0 / C)
            msq_r = vrow.tile([1, TL], FP, name="vr")
            nc.scalar.activation(msq_r[:], pmsq[0:1, :], AF.Identity, scale=1.0 / C)
            var_r = vrow.tile([1, TL], FP, name="vr")
            TT(out=var_r[:], in0=mean_r[:].bitcast(FP), in1=mean_r[:].bitcast(FP),
               op=OP.mult)
            TT(out=var_r[:], in0=msq_r[:], in1=var_r[:], op=OP.subtract)
            TS(out=var_r[:], in0=var_r[:], scalar1=LN_EPS, scalar2=None,
               op0=OP.add)
            sd_r = vrow.tile([1, TL], FP, name="vr")
            nc.scalar.activation(sd_r[:], var_r[:], AF.Sqrt)
            rstd_f = vrow.tile([1, TL], FP, name="vr")
            nc.vector.reciprocal_approx_fast(rstd_f[:], sd_r[:])
            rstd_r = vrow.tile([1, TL], FPR, name="vr")
            nc.scalar.copy(rstd_r[:], rstd_f[:])
            pmean = pscore.tile([P, TL], FP, name="sc")
            prstd = pscore.tile([P, TL], FP, name="sc")
            nc.tensor.matmul(pmean[:], ones1[0:1, :], mean_r[:], start=True, stop=True)
            nc.tensor.matmul(prstd[:], ones1[0:1, :], rstd_r[:],
                             start=True, stop=True)
            pmean_sb = vec.tile([P, TL], FP, name="v")
            prstd_sb = vec.tile([P, TL], FP, name="v")
            nc.scalar.copy(pmean_sb[:], pmean[:])
            nc.scalar.copy(prstd_sb[:], prstd[:])
            for cc in range(CCN):
                t1 = vec.tile([P, TL], FP, name="v")
                TT(out=t1[:], in0=src[:, cc].bitcast(FP),
                   in1=pmean_sb[:], op=OP.subtract)
                TT(out=t1[:], in0=t1[:], in1=prstd_sb[:], op=OP.mult)
                TS(out=dst[:, cc], in0=t1[:], scalar1=lncol(wb, cc),
                   scalar2=lncol(wb + 1, cc), op0=OP.mult, op1=OP.add)

        hist = []  # slot ids of stored residuals, oldest..newest

        for it in range(num_iters):
            s_new = it % MH
            prev = hist[-4:]
            Kn = len(prev)

            zctx = itp.tile([P, CCN, TL], FPR, name="zctx")
            x1 = itp.tile([P, CCN, TL], FPR, name="xln")
            q_sb = itp.tile([P, CCN, TL], FPR, name="qattn")
            k_loc = itp.tile([P, CCN, TL], FPR, name="k_loc")
            k_rem = itp.tile([P, CCN, TL], FPR, name="k_rem")
            # V layout: [kc, head-pair, 130]: even head v at 0:64, ones at
            # col 64 (even head softmax denominator rides the AV matmul as
            # psum row 64), odd head v at 65:129, ones at col 129
            v_loc = itp.tile([P, 4, HPN, 130], FPR, name="v_loc")
            v_rem = itp.tile([P, 4, HPN, 130], FPR, name="v_rem")
            nc.vector.memset(v_loc[:].bitcast(FP), 1.0)
            nc.vector.memset(v_rem[:].bitcast(FP), 1.0)

            # ---- A1: z_ctx = z + u + 0.1*emb_it ; x1 = LN1(z_ctx) ----
            for cc in range(CCN):
                ut = fpool.tile([P, TL], FP, name="ft")
                nc.sync.dma_start(ut[:], u_d[cc * P:(cc + 1) * P, :])
                if it == 0:
                    TS(out=zctx[:, cc], in0=ut[:], scalar1=ecol(it, cc),
                       scalar2=None, op0=OP.add)
                else:
                    t0 = vec.tile([P, TL], FP, name="v")
                    TS(out=t0[:], in0=z_sb[:, cc], scalar1=ecol(it, cc),
                       scalar2=None, op0=OP.add)
                    TT(out=zctx[:, cc], in0=t0[:], in1=ut[:], op=OP.add)

            layernorm(zctx, x1, 0)

            # ---- A2: K (feature-major) and V (token-major) projections ----
            for oc in range(CCN):
                wt = wpool.tile([P, CCN, P], FPR, name="wt")
                nc.sync.dma_start(wt[:], qkw_d[6 + oc])
                pk = pmm.tile([P, TL], FP, name="pg")
                for cc in range(CCN):
                    nc.tensor.matmul(pk[:], wt[:, cc], x1[:, cc],
                                     start=(cc == 0), stop=(cc == CCN - 1))
                nc.scalar.activation(k_loc[:, oc], pk[:], AF.Identity,
                                     bias=bqk_sb[:, 6 + oc:7 + oc])
                nc.sync.dma_start(
                    kvcc[oc * P * TL:(oc + 1) * P * TL],
                    k_loc[:, oc].bitcast(FP))

            vw = big.tile([P, CCN, C], FPR, name="bigt")
            for cc in range(CCN):
                nc.sync.dma_start(vw[:, cc], vw_d[cc])
            for tch in range(4):
                pva = pmm.tile([P, 4, 2, 64], FP, name="pg")
                pvb = pmm.tile([P, 2, 2, 64], FP, name="pg")
                ts = slice(tch * P, (tch + 1) * P)
                for cc in range(CCN):
                    nc.tensor.matmul(pva[:], x1[:, cc, ts], vw[:, cc, 0:512],
                                     start=(cc == 0), stop=False)
                    nc.tensor.matmul(pvb[:], x1[:, cc, ts],
                                     vw[:, cc, 512:768],
                                     start=(cc == 0), stop=False)
                nc.tensor.matmul(pva[:], ones1[0:1, :], vb_sb[:, 0:512],
                                 start=False, stop=True)
                nc.tensor.matmul(pvb[:], ones1[0:1, :], vb_sb[:, 512:768],
                                 start=False, stop=True)
                nc.vector.tensor_copy(v_loc[:, tch, 0:4, 0:64],
                                      pva[:, :, 0, :])
                nc.vector.tensor_copy(v_loc[:, tch, 0:4, 65:129],
                                      pva[:, :, 1, :])
                nc.vector.tensor_copy(v_loc[:, tch, 4:6, 0:64],
                                      pvb[:, :, 0, :])
                nc.vector.tensor_copy(v_loc[:, tch, 4:6, 65:129],
                                      pvb[:, :, 1, :])
                nc.sync.dma_start(
                    kvcc[VOFF + tch * P * 384:VOFF + (tch + 1) * P * 384],
                    v_loc[:, tch, :, 0:64].bitcast(FP))
                nc.sync.dma_start(
                    kvcc[VODD + tch * P * 384:VODD + (tch + 1) * P * 384],
                    v_loc[:, tch, :, 65:129].bitcast(FP))

            nc.gpsimd.collective_compute(
                "AllGather", OP.bypass, replica_groups=GROUPS,
                ins=[kvcc[:]], outs=[kvall[:]])

            # ---- A3: Q projection (overlaps the V collective) ----
            for oc in range(CCN):
                wt = wpool.tile([P, CCN, P], FPR, name="wt")
                nc.sync.dma_start(wt[:], qkw_d[oc])
                pq = pmm.tile([P, TL], FP, name="pg")
                for cc in range(CCN):
                    nc.tensor.matmul(pq[:], wt[:, cc], x1[:, cc],
                                     start=(cc == 0), stop=(cc == CCN - 1))
                nc.scalar.activation(q_sb[:, oc], pq[:], AF.Identity,
                                     bias=bqk_sb[:, oc:oc + 1])

            # remote K/V = gathered slot0 + slot1 - local (rank-agnostic).
            # slot1 staged through the dead zctx/x1 slots; all HW-DGE DMAs.
            ktmp = itp.tile([P, CCN, TL], FPR, name="zctx")
            vtmp = itp.tile([P, 4, 2, HPN, 64], FPR, name="xln")
            for cc in range(CCN):
                nc.sync.dma_start(
                    k_rem[:, cc].bitcast(FP),
                    kvall[0, cc * P * TL:(cc + 1) * P * TL])
                nc.sync.dma_start(
                    ktmp[:, cc].bitcast(FP),
                    kvall[1, cc * P * TL:(cc + 1) * P * TL])
            for tch in range(4):
                nc.sync.dma_start(
                    v_rem[:, tch, :, 0:64].bitcast(FP),
                    kvall[0, VOFF + tch * P * 384:VOFF + (tch + 1) * P * 384])
                nc.sync.dma_start(
                    v_rem[:, tch, :, 65:129].bitcast(FP),
                    kvall[0, VODD + tch * P * 384:VODD + (tch + 1) * P * 384])
                nc.sync.dma_start(
                    vtmp[:, tch, 0].bitcast(FP),
                    kvall[1, VOFF + tch * P * 384:VOFF + (tch + 1) * P * 384])
                nc.sync.dma_start(
                    vtmp[:, tch, 1].bitcast(FP),
                    kvall[1, VODD + tch * P * 384:VODD + (tch + 1) * P * 384])
            TT(out=k_rem[:], in0=k_rem[:].bitcast(FP),
               in1=ktmp[:].bitcast(FP), op=OP.add)
            TT(out=k_rem[:], in0=k_rem[:].bitcast(FP),
               in1=k_loc[:].bitcast(FP), op=OP.subtract)
            for tch in range(4):
                for par, cs in ((0, slice(0, 64)), (1, slice(65, 129))):
                    TT(out=v_rem[:, tch, :, cs],
                       in0=v_rem[:, tch, :, cs].bitcast(FP),
                       in1=vtmp[:, tch, par].bitcast(FP), op=OP.add)
                    TT(out=v_rem[:, tch, :, cs],
                       in0=v_rem[:, tch, :, cs].bitcast(FP),
                       in1=v_loc[:, tch, :, cs].bitcast(FP), op=OP.subtract)

            # ---- B: attention (softmax denominator rides in the AV matmul
            # via the ones columns of v_loc/v_rem) ----
            out_fm = itp.tile([P, CCN, TL], FPR, name="zctx")
            for hp in range(HPN):
                pava = pav.tile([P, TL], FP, name="pv")
                pavb = pav.tile([P, TL], FP, name="pv")
                for kc in range(KCN):
                    if kc < 4:
                        kt, vt, ks = k_loc, v_loc, slice(kc * P, (kc + 1) * P)
                        vkc = kc
                    else:
                        kt, vt = k_rem, v_rem
                        ks = slice((kc - 4) * P, (kc - 3) * P)
                        vkc = kc - 4
                    sca = pscore.tile([P, TL], FP, name="sc")
                    scb = pscore.tile([P, TL], FP, name="sc")
                    nc.tensor.matmul(sca[:], kt[0:64, hp, ks], q_sb[0:64, hp],
                                     start=True, stop=True)
                    nc.tensor.matmul(scb[:], kt[64:128, hp, ks],
                                     q_sb[64:128, hp], start=True, stop=True)
                    atta = vec.tile([P, TL], FPR, name="v")
                    attb = vec.tile([P, TL], FPR, name="v")
                    nc.scalar.activation(atta[:], sca[:], AF.Exp, scale=0.125)
                    nc.scalar.activation(attb[:], scb[:], AF.Exp, scale=0.125)
                    nc.tensor.matmul(pava[0:65, :], vt[:, vkc, hp, 0:65],
                                     atta[:], start=(kc == 0),
                                     stop=(kc == KCN - 1))
                    nc.tensor.matmul(pavb[0:65, :], vt[:, vkc, hp, 65:130],
                                     attb[:], start=(kc == 0),
                                     stop=(kc == KCN - 1))
                rar = vec.tile([P, TL], FPR, name="v")
                rbr = vec.tile([P, TL], FPR, name="v")
                with nc.allow_low_precision(reason="fp32r for PE broadcast"):
                    nc.vector.reciprocal(rar[64:65, :], pava[64:65, :])
                    nc.vector.reciprocal(rbr[64:65, :], pavb[64:65, :])
                pba = pscore.tile([P, TL], FP, name="sc")
                pbb = pscore.tile([P, TL], FP, name="sc")
                nc.tensor.matmul(pba[0:64, :], ones1[64:65, 0:64],
                                 rar[64:65, :],
                                 start=True, stop=True)
                nc.tensor.matmul(pbb[0:64, :], ones1[64:65, 0:64],
                                 rbr[64:65, :], start=True, stop=True)
                bc_sb = vec.tile([P, TL], FP, name="v")
                bcb_sb = vec.tile([P, TL], FP, name="v")
                nc.scalar.copy(bc_sb[0:64, :], pba[0:64, :])
                nc.scalar.copy(bcb_sb[0:64, :], pbb[0:64, :])
                TT(out=out_fm[0:64, hp], in0=pava[0:64, :], in1=bc_sb[0:64, :],
                   op=OP.mult)
                tb = vec.tile([64, TL], FPR, name="vtb")
                TT(out=tb[:], in0=pavb[0:64, :], in1=bcb_sb[0:64, :],
                   op=OP.mult)
                nc.sync.dma_start(out_fm[64:128, hp], tb[:])

            # ---- C: output projection -> attnres (f32) ----
            attnres = itp.tile([P, CCN, TL], FP, name="qattn")
            for oc in range(CCN):
                wt = wpool.tile([P, CCN, P], FPR, name="wt")
                nc.sync.dma_start(wt[:], wo_d[oc])
                pp = pmm.tile([P, TL], FP, name="pg")
                for ci in range(CCN):
                    nc.tensor.matmul(pp[:], wt[:, ci], out_fm[:, ci],
                                     start=(ci == 0), stop=(ci == CCN - 1))
                nc.scalar.activation(attnres[:, oc], pp[:], AF.Identity,
                                     bias=bo_sb[:, oc:oc + 1])

            # ---- D: z_attn = z + attnres ; x2 = LN2(z_attn) ----
            za = itp.tile([P, CCN, TL], FPR, name="zctx")
            for cc in range(CCN):
                if it == 0:
                    nc.vector.tensor_copy(za[:, cc], attnres[:, cc])
                else:
                    TT(out=za[:, cc], in0=z_sb[:, cc], in1=attnres[:, cc],
                       op=OP.add)
            x2 = itp.tile([P, CCN, TL], FPR, name="xln")
            layernorm(za, x2, 2)

            # ---- E: MLP fused per hidden-block; res += mlp into attnres ----
            po = [pmm.tile([P, TL], FP, name="pg"),
                  pmm.tile([P, TL], FP, name="pg"),
                  pscore.tile([P, TL], FP, name="sc"),
                  pscore.tile([P, TL], FP, name="sc"),
                  pav.tile([P, TL], FP, name="pv"),
                  pav.tile([P, TL], FP, name="pv")]
            for hi in range(HCN):
                w1t = wpool.tile([P, CCN, P], FPR, name="wt")
                nc.sync.dma_start(w1t[:], w1_d[hi])
                w2t = w2pool.tile([P, CCN, P], FPR, name="w2t")
                nc.sync.dma_start(w2t[:], w2_d[hi])
                ph = psum2.tile([P, TL], FP, name="p2")
                for cc in range(CCN):
                    nc.tensor.matmul(ph[:], w1t[:, cc], x2[:, cc],
                                     start=(cc == 0), stop=(cc == CCN - 1))
                g = gpool.tile([P, TL], FPR, name="g")
                nc.scalar.activation(g[:], ph[:], AF.Gelu,
                                     bias=b1_sb[:, hi:hi + 1])
                for oc in range(CCN):
                    nc.tensor.matmul(po[oc][:], w2t[:, oc], g[:],
                                     start=(hi == 0), stop=(hi == HCN - 1))
            for oc in range(CCN):
                t2 = vec.tile([P, TL], FP, name="v")
                TS(out=t2[:], in0=po[oc][:],
                   scalar1=b2_sb[:, oc:oc + 1], scalar2=None, op0=OP.add)
                TT(out=attnres[:, oc], in0=attnres[:, oc],
                   in1=t2[:], op=OP.add)

            # store res as newest history entry
            for cc in range(CCN):
                nc.sync.dma_start(fh[s_new, cc * P:(cc + 1) * P, :], attnres[:, cc])

            # ---- F: Anderson update ----
            # raw-dot cache: drows row TRI[(a,b)] = per-token <F_a, F_b>
            # (slots a<=b).  Each iteration adds Kn+1 new dot rows (history
            # slots vs the fresh residual + its self-dot); the Gram matrix /
            # rhs of the per-token least squares is then assembled from
            # cached rows by one constant-matrix f32 matmul (smat).
            pdl_alloc = [(pmm, "pg"), (pmm, "pg"), (pscore, "sc"),
                         (pscore, "sc"), (pav, "pv")]
            pdl = []
            for k in range(Kn + 1):
                pl, nm = pdl_alloc[k]
                pdl.append(pl.tile([2, TL], FP, name=nm))
            for cc in range(CCN):
                for k in range(Kn):
                    ft = fpool.tile([P, TL], FP, name="ft")
                    nc.sync.dma_start(
                        ft[:], fh[prev[k], cc * P:(cc + 1) * P, :])
                    prod = vec.tile([P, TL], FP, name="v")
                    TT(out=prod[:], in0=ft[:], in1=attnres[:, cc], op=OP.mult)
                    nc.tensor.matmul(pdl[k][:], ones2f[:], prod[:],
                                     start=(cc == 0), stop=(cc == CCN - 1))
                sqp = vec.tile([P, TL], FP, name="v")
                nc.scalar.activation(sqp[:], attnres[:, cc], AF.Square)
                nc.tensor.matmul(pdl[Kn][:], ones2f[:], sqp[:],
                                 start=(cc == 0), stop=(cc == CCN - 1))
            for k in range(Kn + 1):
                os = prev[k] if k < Kn else s_new
                row = TRI[(min(os, s_new), max(os, s_new))]
                stage = vrow.tile([1, TL], FP, name="vr")
                nc.scalar.copy(stage[:], pdl[k][0:1, :])
                nc.sync.dma_start(drows[row:row + 1, :], stage[:])

            if Kn == 0:
                for cc in range(CCN):
                    nc.vector.tensor_copy(z_sb[:, cc], attnres[:, cc])
            else:
                # G grid (4x4 incl. symmetric dups) + rhs rows from the
                # dot cache, then transpose to token-major work layout
                gps = psum2.tile([P, TL], FP, name="p2")
                nc.tensor.matmul(gps[0:20, :],
                                 smat_sb[:, it * 20:(it + 1) * 20],
                                 drows[:], start=True, stop=True)
                gsb = vec.tile([P, TL], FP, name="v")
                nc.scalar.copy(gsb[0:20, :], gps[0:20, :])
                for tch in range(4):
                    ptr = pmm.tile([P, TL], FP, name="pg")
                    nc.tensor.transpose(ptr[:, 0:20],
                                        gsb[0:20, tch * P:(tch + 1) * P],
                                        ident[0:20, 0:20])
                    nc.scalar.copy(work[:, tch, 0:20], ptr[:, 0:20])
                for a in range(Kn):
                    TS(out=work[:, :, a * 4 + a], in0=work[:, :, a * 4 + a],
                       scalar1=1e-6, scalar2=None, op0=OP.add)

                def As(a, b):
                    return work[:, :, a * 4 + b]

                def Bs(k):
                    return work[:, :, 16 + k]

                def Al(k):
                    return work[:, :, 20 + k]

                rin = work[:, :, 24]
                tmp = work[:, :, 25]
                fco = work[:, :, 26]
                for i in range(Kn):
                    nc.vector.reciprocal_approx_fast(rin, As(i, i))
                    for j in range(i + 1, Kn):
                        TT(out=fco, in0=As(j, i), in1=rin, op=OP.mult)
                        for m in range(i, Kn):
                            TT(out=tmp, in0=fco, in1=As(i, m), op=OP.mult)
                            TT(out=As(j, m), in0=As(j, m), in1=tmp,
                               op=OP.subtract)
                        TT(out=tmp, in0=fco, in1=Bs(i), op=OP.mult)
                        TT(out=Bs(j), in0=Bs(j), in1=tmp, op=OP.subtract)
                for i in range(Kn - 1, -1, -1):
                    nc.vector.tensor_copy(tmp, Bs(i))
                    for j in range(i + 1, Kn):
                        TT(out=fco, in0=As(i, j), in1=Al(j), op=OP.mult)
                        TT(out=tmp, in0=tmp, in1=fco, op=OP.subtract)
                    nc.vector.reciprocal_approx_fast(rin, As(i, i))
                    TT(out=Al(i), in0=tmp, in1=rin, op=OP.mult)

                # coeffs: col0 = 1 + sum(alpha); cols 1..Kn = -alpha
                if Kn == 1:
                    TS(out=coef_tm[:, :, 0], in0=Al(0), scalar1=1.0,
                       scalar2=None, op0=OP.add)
                else:
                    nc.vector.tensor_reduce(out=coef_tm[:, :, 0:1],
                                            in_=work[:, :, 20:20 + Kn],
                                            axis=AX.X, op=OP.add)
                    TS(out=coef_tm[:, :, 0], in0=coef_tm[:, :, 0],
                       scalar1=1.0, scalar2=None, op0=OP.add)
                TS(out=coef_tm[:, :, 1:1 + Kn], in0=work[:, :, 20:20 + Kn],
                   scalar1=-1.0, scalar2=None, op0=OP.mult)

                for tch in range(4):
                    ptr = pmm.tile([P, TL], FP, name="pg")
                    nc.tensor.transpose(ptr[0:Kn + 1, 0:P],
                                        coef_tm[:, tch, 0:Kn + 1], ident[:])
                    nc.scalar.copy(crows[0:Kn + 1, tch * P:(tch + 1) * P],
                                   ptr[0:Kn + 1, 0:P])

                for k in range(Kn + 1):
                    cst = vrow.tile([1, TL], FPR, name="vrr")
                    nc.sync.dma_start(cst[:], crows[k:k + 1, :])
                    pb = pscore.tile([P, TL], FP, name="sc")
                    nc.tensor.matmul(pb[:], ones1[0:1, :], cst[:],
                                     start=True, stop=True)
                    nc.scalar.copy(coefbc[:, k, :], pb[:])

                # z += c0*res + sum_k ck*F_k
                for cc in range(CCN):
                    t0 = vec.tile([P, TL], FP, name="v")
                    TT(out=t0[:], in0=attnres[:, cc], in1=coefbc[:, 0, :],
                       op=OP.mult)
                    TT(out=z_sb[:, cc], in0=z_sb[:, cc], in1=t0[:], op=OP.add)
                    for k in range(Kn):
                        ft = fpool.tile([P, TL], FP, name="ft")
                        nc.sync.dma_start(
                            ft[:], fh[prev[k], cc * P:(cc + 1) * P, :])
                        TT(out=t0[:], in0=ft[:], in1=coefbc[:, k + 1, :],
                           op=OP.mult)
                        TT(out=z_sb[:, cc], in0=z_sb[:, cc], in1=t0[:],
                           op=OP.add)

            hist.append(s_new)
            if len(hist) > MH:
                hist.pop(0)

        for cc in range(CCN):
            nc.sync.dma_start(zo_d[cc * P:(cc + 1) * P, :], z_sb[:, cc])

        ctx.close()

    nc.finalize()
    return nc


def _host_pack(inputs, num_iters):
    f32 = np.float32
    ipw = np.ascontiguousarray(inputs["in_proj_w"], f32)
    ipb = np.ascontiguousarray(inputs["in_proj_b"], f32)
    opw = np.ascontiguousarray(inputs["out_proj_w"], f32)
    opb = np.ascontiguousarray(inputs["out_proj_b"], f32)
    w1 = np.ascontiguousarray(inputs["mlp_w1"], f32)
    b1 = np.ascontiguousarray(inputs["mlp_b1"], f32)
    w2 = np.ascontiguousarray(inputs["mlp_w2"], f32)
    b2 = np.ascontiguousarray(inputs["mlp_b2"], f32)
    emb = np.ascontiguousarray(inputs["iter_emb"], f32)

    qkw_pack = np.ascontiguousarray(
        ipw[:1536].reshape(12, P, CCN, P).transpose(0, 3, 2, 1))
    vw_pack = np.ascontiguousarray(ipw[1536:].T.reshape(CCN, P, C))
    wo_pack = np.ascontiguousarray(
        opw.reshape(CCN, P, CCN, P).transpose(0, 3, 2, 1))
    w1_pack = np.ascontiguousarray(
        w1.reshape(HCN, P, CCN, P).transpose(0, 3, 2, 1))
    w2t_pack = np.ascontiguousarray(
        w2.T.reshape(HCN, P, CCN, P))
    vbias_row = np.ascontiguousarray(ipb[1536:].reshape(1, C))
    bqk_cols = np.ascontiguousarray(ipb[:1536].reshape(12, P).T)
    bo_cols = np.ascontiguousarray(opb.reshape(CCN, P).T)
    b1_cols = np.ascontiguousarray(b1.reshape(HCN, P).T)
    b2_cols = np.ascontiguousarray(b2.reshape(CCN, P).T)
    ln_cols = np.ascontiguousarray(np.stack(
        [inputs["ln1_w"], inputs["ln1_b"], inputs["ln2_w"], inputs["ln2_b"]],
        0).astype(f32).reshape(4 * CCN, P).T)
    smat_cols = smat_host(num_iters)
    rows = [min(i, emb.shape[0] - 1) for i in range(num_iters)]
    emb_cols = np.ascontiguousarray(
        (0.1 * emb[rows]).reshape(num_iters, CCN, P).transpose(2, 0, 1)
        .reshape(P, num_iters * CCN))
    shared = dict(
        qkw_pack=qkw_pack, vw_pack=vw_pack, wo_pack=wo_pack, w1_pack=w1_pack,
        w2t_pack=w2t_pack, vbias_row=vbias_row, bqk_cols=bqk_cols,
        bo_cols=bo_cols, b1_cols=b1_cols, b2_cols=b2_cols, ln_cols=ln_cols,
        emb_cols=emb_cols, smat_cols=smat_cols)
    u = np.ascontiguousarray(inputs["u"], f32)
    in_maps = []
    for core in range(NCORES):
        b, h = core // 2, core % 2
        m = dict(shared)
        m["u_fm"] = np.ascontiguousarray(u[b, h * TL:(h + 1) * TL, :].T)
        in_maps.append(m)
    return in_maps


def run_device(inputs, num_iters=None, trace=False):
    from concourse.bass_utils import run_bass_kernel_spmd
    ni = int(inputs.get("num_iters", 6)) if num_iters is None else num_iters
    if ni not in _CACHE:
        _CACHE[ni] = _build(ni)
    nc = _CACHE[ni]
    in_maps = _host_pack(inputs, ni)
    r = run_bass_kernel_spmd(nc, in_maps, list(range(NCORES)), trace=trace)
    u = inputs["u"]
    B, T, _ = u.shape
    out = np.empty((B, T, C), np.float32)
    for core in range(NCORES):
        b, h = core // 2, core % 2
        out[b, h * TL:(h + 1) * TL, :] = r.results[core]["z_out"].T
    return out, r


def kernel(**inputs):
    out, _ = run_device(inputs)
    return out.astype(np.float32)



# revision 42
# speedup vs baseline: 1.0208x; 1.0039x over previous
"""DEQ transformer block with Anderson acceleration on 8 Trainium2 NeuronCores.

Sharding: each of the 4 sequences (B=4) is split across a pair of cores
(512 tokens each).  Everything except attention K/V is token-parallel; K/V
halves are exchanged within each pair via AllGather every DEQ iteration.
Activations are feature-major [C, T] in SBUF; matmuls run in fp32r
(full-rate fp32 on the PE, ~13 mantissa bits).
"""

import numpy as np

P = 128
TL = 512          # tokens per core (half a sequence)
C = 768
CCN = 6           # C / 128
NH = 12
DH = 64
HPN = 6           # head pairs
NHID = 3072
HCN = 24          # NHID / 128
KCN = 8           # full-seq key chunks (1024 / 128)
MH = 5            # Anderson history window
LN_EPS = 1e-5
NCORES = 8
GROUPS = [[0, 1], [2, 3], [4, 5], [6, 7]]

_CACHE = {}

TRI = {}
_i = 0
for _a in range(MH):
    for _b in range(_a, MH):
        TRI[(_a, _b)] = _i
        _i += 1


def smat_host(num_iters):
    S = np.zeros((num_iters, 16, 20), np.float32)
    hist = []
    for it in range(num_iters):
        s_new = it % MH
        prev = hist[-4:]
        Kn = len(prev)

        def tri(x, y):
            return TRI[(min(x, y), max(x, y))]

        for a in range(Kn):
            for b in range(Kn):
                col = a * 4 + b
                S[it, tri(prev[a], prev[b]), col] += 1
                S[it, tri(prev[a], s_new), col] -= 1
                S[it, tri(prev[b], s_new), col] -= 1
                S[it, tri(s_new, s_new), col] += 1
            S[it, tri(prev[a], s_new), 16 + a] += 1
            S[it, tri(s_new, s_new), 16 + a] -= 1
        hist.append(s_new)
        if len(hist) > MH:
            hist.pop(0)
    return np.ascontiguousarray(S.transpose(1, 0, 2).reshape(16, num_iters * 20))


def _build(num_iters):
    from contextlib import ExitStack
    import concourse.bass as bass  # noqa
    import concourse.mybir as mybir
    import concourse.tile as tile
    from concourse import bacc
    from concourse.masks import make_identity

    FP = mybir.dt.float32
    FPR = mybir.dt.float32r
    AF = mybir.ActivationFunctionType
    OP = mybir.AluOpType
    AX = mybir.AxisListType

    nc = bacc.Bacc()

    # ---------------- DRAM I/O ----------------
    u_d = nc.dram_tensor("u_fm", [C, TL], FP, kind="ExternalInput")
    qkw_d = nc.dram_tensor("qkw_pack", [12, P, CCN, P], FPR, kind="ExternalInput")
    vw_d = nc.dram_tensor("vw_pack", [CCN, P, C], FPR, kind="ExternalInput")
    wo_d = nc.dram_tensor("wo_pack", [CCN, P, CCN, P], FPR, kind="ExternalInput")
    w1_d = nc.dram_tensor("w1_pack", [HCN, P, CCN, P], FPR, kind="ExternalInput")
    w2_d = nc.dram_tensor("w2t_pack", [HCN, P, CCN, P], FPR, kind="ExternalInput")
    vb_d = nc.dram_tensor("vbias_row", [1, C], FPR, kind="ExternalInput")
    bqk_d = nc.dram_tensor("bqk_cols", [P, 12], FP, kind="ExternalInput")
    bo_d = nc.dram_tensor("bo_cols", [P, CCN], FP, kind="ExternalInput")
    b1_d = nc.dram_tensor("b1_cols", [P, HCN], FP, kind="ExternalInput")
    b2_d = nc.dram_tensor("b2_cols", [P, CCN], FP, kind="ExternalInput")
    ln_d = nc.dram_tensor("ln_cols", [P, 4 * CCN], FP, kind="ExternalInput")
    emb_d = nc.dram_tensor("emb_cols", [P, num_iters * CCN], FP, kind="ExternalInput")
    smat_d = nc.dram_tensor("smat_cols", [16, num_iters * 20], FP,
                            kind="ExternalInput")
    zo_d = nc.dram_tensor("z_out", [C, TL], FP, kind="ExternalOutput")

    # internal DRAM
    # combined K+V exchange buffer (flat): K at [0, C*TL) feature-major,
    # V-even-heads at [VOFF, +TL*384), V-odd-heads at [VOFF2, +TL*384)
    KVN = C * TL + TL * C
    VOFF = C * TL
    VODD = VOFF + TL * HPN * 64
    kvcc = nc.dram_tensor("kv_cc", [KVN], FP)
    kvall = nc.dram_tensor("kv_all", [2, KVN], FP)
    fh = nc.dram_tensor("f_hist", [MH, C, TL], FP)

    with tile.TileContext(nc) as tc:
        ctx = ExitStack()
        pool = ctx.enter_context(tc.tile_pool(name="pers", bufs=1))
        vec = ctx.enter_context(tc.tile_pool(name="vec", bufs=6))
        vrow = ctx.enter_context(tc.tile_pool(name="vrow", bufs=4))
        wpool = ctx.enter_context(tc.tile_pool(name="wpool", bufs=4))
        w2pool = ctx.enter_context(tc.tile_pool(name="w2pool", bufs=2))
        gpool = ctx.enter_context(tc.tile_pool(name="gpool", bufs=2))
        fpool = ctx.enter_context(tc.tile_pool(name="fpool", bufs=4))
        big = ctx.enter_context(tc.tile_pool(name="bigp", bufs=1))
        itp = ctx.enter_context(tc.tile_pool(name="itp", bufs=1))
        pmm = ctx.enter_context(tc.tile_pool(name="pmm", bufs=2, space="PSUM"))
        pscore = ctx.enter_context(tc.tile_pool(name="pscore", bufs=2, space="PSUM"))
        pav = ctx.enter_context(tc.tile_pool(name="pav", bufs=2, space="PSUM"))
        psum2 = ctx.enter_context(tc.tile_pool(name="psum2", bufs=2, space="PSUM"))

        # ------------- persistent tiles -------------
        z_sb = pool.tile([P, CCN, TL], FP, name="z_sb")
        bqk_sb = pool.tile([P, 12], FP, name="bqk_sb")
        bo_sb = pool.tile([P, CCN], FP, name="bo_sb")
        b1_sb = pool.tile([P, HCN], FP, name="b1_sb")
        b2_sb = pool.tile([P, CCN], FP, name="b2_sb")
        ln_sb = pool.tile([P, 4 * CCN], FP, name="ln_sb")
        emb_sb = pool.tile([P, num_iters * CCN], FP, name="emb_sb")
        vb_sb = pool.tile([1, C], FPR, name="vb_sb")
        ident = pool.tile([P, P], FP, name="ident")
        ones1 = pool.tile([P, P], FPR, name="ones1")
        ones2 = pool.tile([P, 2], FPR, name="ones2")
        ones2f = pool.tile([P, 2], FP, name="ones2f")
        coefbc = pool.tile([P, MH, TL], FP, name="coefbc")
        drows = pool.tile([16, TL], FP, name="drows")
        work = pool.tile([P, 4, 28], FP, name="work")
        coef_tm = pool.tile([P, 4, MH], FP, name="coef_tm")
        smat_sb = pool.tile([16, num_iters * 20], FP, name="smat_sb")
        crows = pool.tile([8, TL], FPR, name="crows")

        nc.sync.dma_start(bqk_sb[:], bqk_d[:])
        nc.sync.dma_start(bo_sb[:], bo_d[:])
        nc.sync.dma_start(b1_sb[:], b1_d[:])
        nc.sync.dma_start(b2_sb[:], b2_d[:])
        nc.sync.dma_start(ln_sb[:], ln_d[:])
        nc.sync.dma_start(emb_sb[:], emb_d[:])
        nc.sync.dma_start(vb_sb[:], vb_d[:])
        nc.sync.dma_start(smat_sb[:], smat_d[:])
        make_identity(nc, ident[:])
        nc.vector.memset(drows[:], 0.0)
        onesf = vec.tile([P, P], FP, name="v")
        nc.vector.memset(onesf[:], 1.0)
        nc.scalar.copy(ones1[:], onesf[:])
        nc.scalar.copy(ones2[:], onesf[:, 0:2])
        nc.scalar.copy(ones2f[:], onesf[:, 0:2])

        def ecol(it, cc):
            return emb_sb[:, it * CCN + cc:it * CCN + cc + 1]

        def lncol(which, cc):
            return ln_sb[:, which * CCN + cc:which * CCN + cc + 1]

        TT = nc.vector.tensor_tensor
        TS = nc.vector.tensor_scalar

        def layernorm(src, dst, wb):
            # src/dst: [P, CCN, TL] FPR tiles; wb: 0 for ln1, 2 for ln2
            pmu = pmm.tile([2, TL], FP, name="pg")
            pmsq = pmm.tile([2, TL], FP, name="pg")
            for cc in range(CCN):
                sq = vec.tile([P, TL], FPR, name="v")
                nc.scalar.activation(sq[:], src[:, cc].bitcast(FP), AF.Square)
                nc.tensor.matmul(pmu[:], ones2[:], src[:, cc],
                                 start=(cc == 0), stop=(cc == CCN - 1))
                nc.tensor.matmul(pmsq[:], ones2[:], sq[:],
                                 start=(cc == 0), stop=(cc == CCN - 1))
            mean_r = vrow.tile([1, TL], FPR, name="vr")
            nc.scalar.activation(mean_r[:], pmu[0:1, :], AF.Identity, scale=1.0 / C)
            msq_r = vrow.tile([1, TL], FP, name="vr")
            nc.scalar.activation(msq_r[:], pmsq[0:1, :], AF.Identity, scale=1.0 / C)
            var_r = vrow.tile([1, TL], FP, name="vr")
            TT(out=var_r[:], in0=mean_r[:].bitcast(FP), in1=mean_r[:].bitcast(FP),
               op=OP.mult)
            TT(out=var_r[:], in0=msq_r[:], in1=var_r[:], op=OP.subtract)
            TS(out=var_r[:], in0=var_r[:], scalar1=LN_EPS, scalar2=None,
               op0=OP.add)
            sd_r = vrow.tile([1, TL], FP, name="vr")
            nc.scalar.activation(sd_r[:], var_r[:], AF.Sqrt)
            rstd_f = vrow.tile([1, TL], FP, name="vr")
            nc.vector.reciprocal_approx_fast(rstd_f[:], sd_r[:])
            rstd_r = vrow.tile([1, TL], FPR, name="vr")
            nc.scalar.copy(rstd_r[:], rstd_f[:])
            pmean = pscore.tile([P, TL], FP, name="sc")
            prstd = pscore.tile([P, TL], FP, name="sc")
            nc.tensor.matmul(pmean[:], ones1[0:1, :], mean_r[:], start=True, stop=True)
            nc.tensor.matmul(prstd[:], ones1[0:1, :], rstd_r[:],
                             start=True, stop=True)
            pmean_sb = vec.tile([P, TL], FP, name="v")
            prstd_sb = vec.tile([P, TL], FP, name="v")
            nc.scalar.copy(pmean_sb[:], pmean[:])
            nc.scalar.copy(prstd_sb[:], prstd[:])
            for cc in range(CCN):
                t1 = vec.tile([P, TL], FP, name="v")
                TT(out=t1[:], in0=src[:, cc].bitcast(FP),
                   in1=pmean_sb[:], op=OP.subtract)
                TT(out=t1[:], in0=t1[:], in1=prstd_sb[:], op=OP.mult)
                TS(out=dst[:, cc], in0=t1[:], scalar1=lncol(wb, cc),
                   scalar2=lncol(wb + 1, cc), op0=OP.mult, op1=OP.add)

        hist = []  # slot ids of stored residuals, oldest..newest

        for it in range(num_iters):
            s_new = it % MH
            prev = hist[-4:]
            Kn = len(prev)

            zctx = itp.tile([P, CCN, TL], FPR, name="zctx")
            x1 = itp.tile([P, CCN, TL], FPR, name="xln")
            q_sb = itp.tile([P, CCN, TL], FPR, name="qattn")
            k_loc = itp.tile([P, CCN, TL], FPR, name="k_loc")
            k_rem = itp.tile([P, CCN, TL], FPR, name="k_rem")
            # V layout: [kc, head-pair, 130]: even head v at 0:64, ones at
            # col 64 (even head softmax denominator rides the AV matmul as
            # psum row 64), odd head v at 65:129, ones at col 129
            v_loc = itp.tile([P, 4, HPN, 130], FPR, name="v_loc")
            v_rem = itp.tile([P, 4, HPN, 130], FPR, name="v_rem")
            nc.vector.memset(v_loc[:].bitcast(FP), 1.0)
            nc.vector.memset(v_rem[:].bitcast(FP), 1.0)

            # ---- A1: z_ctx = z + u + 0.1*emb_it ; x1 = LN1(z_ctx) ----
            for cc in range(CCN):
                ut = fpool.tile([P, TL], FP, name="ft")
                nc.sync.dma_start(ut[:], u_d[cc * P:(cc + 1) * P, :])
                if it == 0:
                    TS(out=zctx[:, cc], in0=ut[:], scalar1=ecol(it, cc),
                       scalar2=None, op0=OP.add)
                else:
                    t0 = vec.tile([P, TL], FP, name="v")
                    TS(out=t0[:], in0=z_sb[:, cc], scalar1=ecol(it, cc),
                       scalar2=None, op0=OP.add)
                    TT(out=zctx[:, cc], in0=t0[:], in1=ut[:], op=OP.add)

            layernorm(zctx, x1, 0)

            # ---- A2: K (feature-major) and V (token-major) projections ----
            for oc in range(CCN):
                wt = wpool.tile([P, CCN, P], FPR, name="wt")
                nc.sync.dma_start(wt[:], qkw_d[6 + oc])
                pk = pmm.tile([P, TL], FP, name="pg")
                for cc in range(CCN):
                    nc.tensor.matmul(pk[:], wt[:, cc], x1[:, cc],
                                     start=(cc == 0), stop=(cc == CCN - 1))
                nc.scalar.activation(k_loc[:, oc], pk[:], AF.Identity,
                                     bias=bqk_sb[:, 6 + oc:7 + oc])
                nc.sync.dma_start(
                    kvcc[oc * P * TL:(oc + 1) * P * TL],
                    k_loc[:, oc].bitcast(FP))

            vw = big.tile([P, CCN, C], FPR, name="bigt")
            for cc in range(CCN):
                nc.sync.dma_start(vw[:, cc], vw_d[cc])
            for tch in range(4):
                pva = pmm.tile([P, 4, 2, 64], FP, name="pg")
                pvb = pmm.tile([P, 2, 2, 64], FP, name="pg")
                ts = slice(tch * P, (tch + 1) * P)
                for cc in range(CCN):
                    nc.tensor.matmul(pva[:], x1[:, cc, ts], vw[:, cc, 0:512],
                                     start=(cc == 0), stop=False)
                    nc.tensor.matmul(pvb[:], x1[:, cc, ts],
                                     vw[:, cc, 512:768],
                                     start=(cc == 0), stop=False)
                nc.tensor.matmul(pva[:], ones1[0:1, :], vb_sb[:, 0:512],
                                 start=False, stop=True)
                nc.tensor.matmul(pvb[:], ones1[0:1, :], vb_sb[:, 512:768],
                                 start=False, stop=True)
                nc.vector.tensor_copy(v_loc[:, tch, 0:4, 0:64],
                                      pva[:, :, 0, :])
                nc.vector.tensor_copy(v_loc[:, tch, 0:4, 65:129],
                                      pva[:, :, 1, :])
                nc.vector.tensor_copy(v_loc[:, tch, 4:6, 0:64],
                                      pvb[:, :, 0, :])
                nc.vector.tensor_copy(v_loc[:, tch, 4:6, 65:129],
                                      pvb[:, :, 1, :])
                nc.sync.dma_start(
                    kvcc[VOFF + tch * P * 384:VOFF + (tch + 1) * P * 384],
                    v_loc[:, tch, :, 0:64].bitcast(FP))
                nc.sync.dma_start(
                    kvcc[VODD + tch * P * 384:VODD + (tch + 1) * P * 384],
                    v_loc[:, tch, :, 65:129].bitcast(FP))

            nc.gpsimd.collective_compute(
                "AllGather", OP.bypass, replica_groups=GROUPS,
                ins=[kvcc[:]], outs=[kvall[:]])

            # ---- A3: Q projection (overlaps the V collective) ----
            for oc in range(CCN):
                wt = wpool.tile([P, CCN, P], FPR, name="wt")
                nc.sync.dma_start(wt[:], qkw_d[oc])
                pq = pmm.tile([P, TL], FP, name="pg")
                for cc in range(CCN):
                    nc.tensor.matmul(pq[:], wt[:, cc], x1[:, cc],
                                     start=(cc == 0), stop=(cc == CCN - 1))
                nc.scalar.activation(q_sb[:, oc], pq[:], AF.Identity,
                                     bias=bqk_sb[:, oc:oc + 1])

            # remote K/V = gathered slot0 + slot1 - local (rank-agnostic).
            # slot1 staged through the dead zctx/x1 slots; all HW-DGE DMAs.
            ktmp = itp.tile([P, CCN, TL], FPR, name="zctx")
            vtmp = itp.tile([P, 4, 2, HPN, 64], FPR, name="xln")
            for cc in range(CCN):
                nc.sync.dma_start(
                    k_rem[:, cc].bitcast(FP),
                    kvall[0, cc * P * TL:(cc + 1) * P * TL])
                nc.sync.dma_start(
                    ktmp[:, cc].bitcast(FP),
                    kvall[1, cc * P * TL:(cc + 1) * P * TL])
            for tch in range(4):
                nc.sync.dma_start(
                    v_rem[:, tch, :, 0:64].bitcast(FP),
                    kvall[0, VOFF + tch * P * 384:VOFF + (tch + 1) * P * 384])
                nc.sync.dma_start(
                    v_rem[:, tch, :, 65:129].bitcast(FP),
                    kvall[0, VODD + tch * P * 384:VODD + (tch + 1) * P * 384])
                nc.sync.dma_start(
                    vtmp[:, tch, 0].bitcast(FP),
                    kvall[1, VOFF + tch * P * 384:VOFF + (tch + 1) * P * 384])
                nc.sync.dma_start(
                    vtmp[:, tch, 1].bitcast(FP),
                    kvall[1, VODD + tch * P * 384:VODD + (tch + 1) * P * 384])
            TT(out=k_rem[:], in0=k_rem[:].bitcast(FP),
               in1=ktmp[:].bitcast(FP), op=OP.add)
            TT(out=k_rem[:], in0=k_rem[:].bitcast(FP),
               in1=k_loc[:].bitcast(FP), op=OP.subtract)
            for tch in range(4):
                for par, cs in ((0, slice(0, 64)), (1, slice(65, 129))):
                    TT(out=v_rem[:, tch, :, cs],
                       in0=v_rem[:, tch, :, cs].bitcast(FP),
                       in1=vtmp[:, tch, par].bitcast(FP), op=OP.add)
                    TT(out=v_rem[:, tch, :, cs],
                       in0=v_rem[:, tch, :, cs].bitcast(FP),
                       in1=v_loc[:, tch, :, cs].bitcast(FP), op=OP.subtract)

            # ---- B: attention (softmax denominator rides in the AV matmul
            # via the ones columns of v_loc/v_rem) ----
            out_fm = itp.tile([P, CCN, TL], FPR, name="zctx")
            for hp in range(HPN):
                pava = pav.tile([P, TL], FP, name="pv")
                pavb = pav.tile([P, TL], FP, name="pv")
                for kc in range(KCN):
                    if kc < 4:
                        kt, vt = k_loc, v_loc
                        ks = slice(kc * P, (kc + 1) * P)
                        vkc = kc
                    else:
                        kt, vt = k_rem, v_rem
                        ks = slice((kc - 4) * P, (kc - 3) * P)
                        vkc = kc - 4
                    sca = pscore.tile([P, TL], FP, name="sc")
                    scb = pscore.tile([P, TL], FP, name="sc")
                    nc.tensor.matmul(sca[:], kt[0:64, hp, ks], q_sb[0:64, hp],
                                     start=True, stop=True)
                    nc.tensor.matmul(scb[:], kt[64:128, hp, ks],
                                     q_sb[64:128, hp], start=True, stop=True)
                    atta = vec.tile([P, TL], FPR, name="v")
                    attb = vec.tile([P, TL], FPR, name="v")
                    nc.scalar.activation(atta[:], sca[:], AF.Exp, scale=0.125)
                    nc.scalar.activation(attb[:], scb[:], AF.Exp, scale=0.125)
                    nc.tensor.matmul(pava[0:65, :], vt[:, vkc, hp, 0:65],
                                     atta[:], start=(kc == 0),
                                     stop=(kc == KCN - 1))
                    nc.tensor.matmul(pavb[0:65, :], vt[:, vkc, hp, 65:130],
                                     attb[:], start=(kc == 0),
                                     stop=(kc == KCN - 1))
                rar = vec.tile([P, TL], FPR, name="v")
                rbr = vec.tile([P, TL], FPR, name="v")
                with nc.allow_low_precision(reason="fp32r for PE broadcast"):
                    nc.vector.reciprocal(rar[64:65, :], pava[64:65, :])
                    nc.vector.reciprocal(rbr[64:65, :], pavb[64:65, :])
                pba = pscore.tile([P, TL], FP, name="sc")
                pbb = pscore.tile([P, TL], FP, name="sc")
                nc.tensor.matmul(pba[0:64, :], ones1[64:65, 0:64],
                                 rar[64:65, :], start=True, stop=True)
                nc.tensor.matmul(pbb[0:64, :], ones1[64:65, 0:64],
                                 rbr[64:65, :], start=True, stop=True)
                bc_sb = vec.tile([P, TL], FP, name="v")
                bcb_sb = vec.tile([P, TL], FP, name="v")
                nc.scalar.copy(bc_sb[0:64, :], pba[0:64, :])
                nc.scalar.copy(bcb_sb[0:64, :], pbb[0:64, :])
                TT(out=out_fm[0:64, hp], in0=pava[0:64, :],
                   in1=bc_sb[0:64, :], op=OP.mult)
                tb = vec.tile([64, TL], FPR, name="vtb")
                TT(out=tb[:], in0=pavb[0:64, :], in1=bcb_sb[0:64, :],
                   op=OP.mult)
                nc.sync.dma_start(out_fm[64:128, hp], tb[:])

            # ---- C: output projection -> attnres (f32) ----
            attnres = itp.tile([P, CCN, TL], FP, name="qattn")
            for oc in range(CCN):
                wt = wpool.tile([P, CCN, P], FPR, name="wt")
                nc.sync.dma_start(wt[:], wo_d[oc])
                pp = pmm.tile([P, TL], FP, name="pg")
                for ci in range(CCN):
                    nc.tensor.matmul(pp[:], wt[:, ci], out_fm[:, ci],
                                     start=(ci == 0), stop=(ci == CCN - 1))
                nc.scalar.activation(attnres[:, oc], pp[:], AF.Identity,
                                     bias=bo_sb[:, oc:oc + 1])

            # ---- D: z_attn = z + attnres ; x2 = LN2(z_attn) ----
            za = itp.tile([P, CCN, TL], FPR, name="zctx")
            for cc in range(CCN):
                if it == 0:
                    nc.vector.tensor_copy(za[:, cc], attnres[:, cc])
                else:
                    TT(out=za[:, cc], in0=z_sb[:, cc], in1=attnres[:, cc],
                       op=OP.add)
            x2 = itp.tile([P, CCN, TL], FPR, name="xln")
            layernorm(za, x2, 2)

            # ---- E: MLP fused per hidden-block; res += mlp into attnres ----
            po = [pmm.tile([P, TL], FP, name="pg"),
                  pmm.tile([P, TL], FP, name="pg"),
                  pscore.tile([P, TL], FP, name="sc"),
                  pscore.tile([P, TL], FP, name="sc"),
                  pav.tile([P, TL], FP, name="pv"),
                  pav.tile([P, TL], FP, name="pv")]
            for hi in range(HCN):
                w1t = wpool.tile([P, CCN, P], FPR, name="wt")
                nc.sync.dma_start(w1t[:], w1_d[hi])
                w2t = w2pool.tile([P, CCN, P], FPR, name="w2t")
                nc.sync.dma_start(w2t[:], w2_d[hi])
                ph = psum2.tile([P, TL], FP, name="p2")
                for cc in range(CCN):
                    nc.tensor.matmul(ph[:], w1t[:, cc], x2[:, cc],
                                     start=(cc == 0), stop=(cc == CCN - 1))
                g = gpool.tile([P, TL], FPR, name="g")
                nc.scalar.activation(g[:], ph[:], AF.Gelu,
                                     bias=b1_sb[:, hi:hi + 1])
                for oc in range(CCN):
                    nc.tensor.matmul(po[oc][:], w2t[:, oc], g[:],
                                     start=(hi == 0), stop=(hi == HCN - 1))
            for oc in range(CCN):
                t2 = vec.tile([P, TL], FP, name="v")
                TS(out=t2[:], in0=po[oc][:],
                   scalar1=b2_sb[:, oc:oc + 1], scalar2=None, op0=OP.add)
                TT(out=attnres[:, oc], in0=attnres[:, oc],
                   in1=t2[:], op=OP.add)

            # store res as newest history entry
            for cc in range(CCN):
                nc.sync.dma_start(fh[s_new, cc * P:(cc + 1) * P, :], attnres[:, cc])

            # ---- F: Anderson update ----
            # raw-dot cache: drows row TRI[(a,b)] = per-token <F_a, F_b>
            # (slots a<=b).  Each iteration adds Kn+1 new dot rows (history
            # slots vs the fresh residual + its self-dot); the Gram matrix /
            # rhs of the per-token least squares is then assembled from
            # cached rows by one constant-matrix f32 matmul (smat).
            pdl_alloc = [(pmm, "pg"), (pmm, "pg"), (pscore, "sc"),
                         (pscore, "sc"), (pav, "pv")]
            pdl = []
            for k in range(Kn + 1):
                pl, nm = pdl_alloc[k]
                pdl.append(pl.tile([2, TL], FP, name=nm))
            for cc in range(CCN):
                for k in range(Kn):
                    ft = fpool.tile([P, TL], FP, name="ft")
                    nc.sync.dma_start(
                        ft[:], fh[prev[k], cc * P:(cc + 1) * P, :])
                    prod = vec.tile([P, TL], FP, name="v")
                    TT(out=prod[:], in0=ft[:], in1=attnres[:, cc], op=OP.mult)
                    nc.tensor.matmul(pdl[k][:], ones2f[:], prod[:],
                                     start=(cc == 0), stop=(cc == CCN - 1))
                sqp = vec.tile([P, TL], FP, name="v")
                nc.scalar.activation(sqp[:], attnres[:, cc], AF.Square)
                nc.tensor.matmul(pdl[Kn][:], ones2f[:], sqp[:],
                                 start=(cc == 0), stop=(cc == CCN - 1))
            for k in range(Kn + 1):
                os = prev[k] if k < Kn else s_new
                row = TRI[(min(os, s_new), max(os, s_new))]
                stage = vrow.tile([1, TL], FP, name="vr")
                nc.scalar.copy(stage[:], pdl[k][0:1, :])
                nc.sync.dma_start(drows[row:row + 1, :], stage[:])

            if Kn == 0:
                for cc in range(CCN):
                    nc.vector.tensor_copy(z_sb[:, cc], attnres[:, cc])
            else:
                # G grid (4x4 incl. symmetric dups) + rhs rows from the
                # dot cache, then transpose to token-major work layout
                gps = psum2.tile([P, TL], FP, name="p2")
                nc.tensor.matmul(gps[0:20, :],
                                 smat_sb[:, it * 20:(it + 1) * 20],
                                 drows[:], start=True, stop=True)
                gsb = vec.tile([P, TL], FP, name="v")
                nc.scalar.copy(gsb[0:20, :], gps[0:20, :])
                for tch in range(4):
                    ptr = pmm.tile([P, TL], FP, name="pg")
                    nc.tensor.transpose(ptr[:, 0:20],
                                        gsb[0:20, tch * P:(tch + 1) * P],
                                        ident[0:20, 0:20])
                    nc.scalar.copy(work[:, tch, 0:20], ptr[:, 0:20])
                for a in range(Kn):
                    TS(out=work[:, :, a * 4 + a], in0=work[:, :, a * 4 + a],
                       scalar1=1e-6, scalar2=None, op0=OP.add)

                def As(a, b):
                    return work[:, :, a * 4 + b]

                def Bs(k):
                    return work[:, :, 16 + k]

                def Al(k):
                    return work[:, :, 20 + k]

                rin = work[:, :, 24]
                tmp = work[:, :, 25]
                fco = work[:, :, 26]
                for i in range(Kn):
                    nc.vector.reciprocal_approx_fast(rin, As(i, i))
                    for j in range(i + 1, Kn):
                        TT(out=fco, in0=As(j, i), in1=rin, op=OP.mult)
                        for m in range(i, Kn):
                            TT(out=tmp, in0=fco, in1=As(i, m), op=OP.mult)
                            TT(out=As(j, m), in0=As(j, m), in1=tmp,
                               op=OP.subtract)
                        TT(out=tmp, in0=fco, in1=Bs(i), op=OP.mult)
                        TT(out=Bs(j), in0=Bs(j), in1=tmp, op=OP.subtract)
                for i in range(Kn - 1, -1, -1):
                    nc.vector.tensor_copy(tmp, Bs(i))
                    for j in range(i + 1, Kn):
                        TT(out=fco, in0=As(i, j), in1=Al(j), op=OP.mult)
                        TT(out=tmp, in0=tmp, in1=fco, op=OP.subtract)
                    nc.vector.reciprocal_approx_fast(rin, As(i, i))
                    TT(out=Al(i), in0=tmp, in1=rin, op=OP.mult)

                # coeffs: col0 = 1 + sum(alpha); cols 1..Kn = -alpha
                if Kn == 1:
                    TS(out=coef_tm[:, :, 0], in0=Al(0), scalar1=1.0,
                       scalar2=None, op0=OP.add)
                else:
                    nc.vector.tensor_reduce(out=coef_tm[:, :, 0:1],
                                            in_=work[:, :, 20:20 + Kn],
                                            axis=AX.X, op=OP.add)
                    TS(out=coef_tm[:, :, 0], in0=coef_tm[:, :, 0],
                       scalar1=1.0, scalar2=None, op0=OP.add)
                TS(out=coef_tm[:, :, 1:1 + Kn], in0=work[:, :, 20:20 + Kn],
                   scalar1=-1.0, scalar2=None, op0=OP.mult)

                for tch in range(4):
                    ptr = pmm.tile([P, TL], FP, name="pg")
                    nc.tensor.transpose(ptr[0:Kn + 1, 0:P],
                                        coef_tm[:, tch, 0:Kn + 1], ident[:])
                    nc.scalar.copy(crows[0:Kn + 1, tch * P:(tch + 1) * P],
                                   ptr[0:Kn + 1, 0:P])

                for k in range(Kn + 1):
                    cst = vrow.tile([1, TL], FPR, name="vrr")
                    nc.sync.dma_start(cst[:], crows[k:k + 1, :])
                    pb = pscore.tile([P, TL], FP, name="sc")
                    nc.tensor.matmul(pb[:], ones1[0:1, :], cst[:],
                                     start=True, stop=True)
                    nc.scalar.copy(coefbc[:, k, :], pb[:])

                # z += c0*res + sum_k ck*F_k
                for cc in range(CCN):
                    t0 = vec.tile([P, TL], FP, name="v")
                    TT(out=t0[:], in0=attnres[:, cc], in1=coefbc[:, 0, :],
                       op=OP.mult)
                    TT(out=z_sb[:, cc], in0=z_sb[:, cc], in1=t0[:], op=OP.add)
                    for k in range(Kn):
                        ft = fpool.tile([P, TL], FP, name="ft")
                        nc.sync.dma_start(
                            ft[:], fh[prev[k], cc * P:(cc + 1) * P, :])
                        TT(out=t0[:], in0=ft[:], in1=coefbc[:, k + 1, :],
                           op=OP.mult)
                        TT(out=z_sb[:, cc], in0=z_sb[:, cc], in1=t0[:],
                           op=OP.add)

            hist.append(s_new)
            if len(hist) > MH:
                hist.pop(0)

        for cc in range(CCN):
            nc.sync.dma_start(zo_d[cc * P:(cc + 1) * P, :], z_sb[:, cc])

        ctx.close()

    nc.finalize()
    return nc


def _host_pack(inputs, num_iters):
    f32 = np.float32
    ipw = np.ascontiguousarray(inputs["in_proj_w"], f32)
    ipb = np.ascontiguousarray(inputs["in_proj_b"], f32)
    opw = np.ascontiguousarray(inputs["out_proj_w"], f32)
    opb = np.ascontiguousarray(inputs["out_proj_b"], f32)
    w1 = np.ascontiguousarray(inputs["mlp_w1"], f32)
    b1 = np.ascontiguousarray(inputs["mlp_b1"], f32)
    w2 = np.ascontiguousarray(inputs["mlp_w2"], f32)
    b2 = np.ascontiguousarray(inputs["mlp_b2"], f32)
    emb = np.ascontiguousarray(inputs["iter_emb"], f32)

    qkw_pack = np.ascontiguousarray(
        ipw[:1536].reshape(12, P, CCN, P).transpose(0, 3, 2, 1))
    vw_pack = np.ascontiguousarray(ipw[1536:].T.reshape(CCN, P, C))
    wo_pack = np.ascontiguousarray(
        opw.reshape(CCN, P, CCN, P).transpose(0, 3, 2, 1))
    w1_pack = np.ascontiguousarray(
        w1.reshape(HCN, P, CCN, P).transpose(0, 3, 2, 1))
    w2t_pack = np.ascontiguousarray(
        w2.T.reshape(HCN, P, CCN, P))
    vbias_row = np.ascontiguousarray(ipb[1536:].reshape(1, C))
    bqk_cols = np.ascontiguousarray(ipb[:1536].reshape(12, P).T)
    bo_cols = np.ascontiguousarray(opb.reshape(CCN, P).T)
    b1_cols = np.ascontiguousarray(b1.reshape(HCN, P).T)
    b2_cols = np.ascontiguousarray(b2.reshape(CCN, P).T)
    ln_cols = np.ascontiguousarray(np.stack(
        [inputs["ln1_w"], inputs["ln1_b"], inputs["ln2_w"], inputs["ln2_b"]],
        0).astype(f32).reshape(4 * CCN, P).T)
    smat_cols = smat_host(num_iters)
    rows = [min(i, emb.shape[0] - 1) for i in range(num_iters)]
    emb_cols = np.ascontiguousarray(
        (0.1 * emb[rows]).reshape(num_iters, CCN, P).transpose(2, 0, 1)
        .reshape(P, num_iters * CCN))
    shared = dict(
        qkw_pack=qkw_pack, vw_pack=vw_pack, wo_pack=wo_pack, w1_pack=w1_pack,
        w2t_pack=w2t_pack, vbias_row=vbias_row, bqk_cols=bqk_cols,
        bo_cols=bo_cols, b1_cols=b1_cols, b2_cols=b2_cols, ln_cols=ln_cols,
        emb_cols=emb_cols, smat_cols=smat_cols)
    u = np.ascontiguousarray(inputs["u"], f32)
    in_maps = []
    for core in range(NCORES):
        b, h = core // 2, core % 2
        m = dict(shared)
        m["u_fm"] = np.ascontiguousarray(u[b, h * TL:(h + 1) * TL, :].T)
        in_maps.append(m)
    return in_maps


def run_device(inputs, num_iters=None, trace=False):
    from concourse.bass_utils import run_bass_kernel_spmd
    ni = int(inputs.get("num_iters", 6)) if num_iters is None else num_iters
    if ni not in _CACHE:
        _CACHE[ni] = _build(ni)
    nc = _CACHE[ni]
    in_maps = _host_pack(inputs, ni)
    r = run_bass_kernel_spmd(nc, in_maps, list(range(NCORES)), trace=trace)
    u = inputs["u"]
    B, T, _ = u.shape
    out = np.empty((B, T, C), np.float32)
    for core in range(NCORES):
        b, h = core // 2, core % 2
        out[b, h * TL:(h + 1) * TL, :] = r.results[core]["z_out"].T
    return out, r


def kernel(**inputs):
    out, _ = run_device(inputs)
    return out.astype(np.float32)

